# revision 3
# baseline (speedup 1.0000x reference)
"""Distributed causal multi-head attention for one TRN2 chip (8 NeuronCores).

Sharding: batch (2) x head-groups (4 heads/core) -> 8 cores.
Core c handles batch c//4, heads [ (c%4)*4 , (c%4)*4+4 ).
Per core: QKV projections for its 4 heads, flash-style causal attention
with scores kept transposed (S^T = K @ Q^T) so the PV product needs no
transposes; V is augmented with a ones column so the softmax denominators
fall out of the same matmul (row 64 of each head's O^T psum).  Then an
AllGather of the attention output (pre-Wo, 4-core group = one batch) and
a column-sliced output projection.  Host assembles the 8 column/batch
shards.  Compute dtype bf16 (PSUM accumulation fp32), softmax in fp32.
"""

import sys
import numpy as np

sys.path.insert(0, "/opt/trn_rl_repo")

import concourse.bass as bass  # noqa: E402
import concourse.bacc as bacc  # noqa: E402
import concourse.tile as tile  # noqa: E402
import concourse.mybir as mybir  # noqa: E402

F32 = mybir.dt.float32
BF16 = mybir.dt.bfloat16

P = 128          # partition dim
CHUNK = 512      # i-chunk (matmul moving free dim)
DH = 64          # head dim
HPC = 4          # heads per core
HS = HPC * DH    # 256 per-core inner slice
DHA = DH + 1     # augmented head dim (ones column for softmax sums)
INNER = 1024     # total inner dim (16 heads x 64)
N_CORES = 8
GROUPS = [[0, 1, 2, 3], [4, 5, 6, 7]]


def build_nc(seq=2048, dim=1024, n_cores=N_CORES, groups=GROUPS, compile=True):
    """Build the SPMD Bass graph (identical on all cores)."""
    nch = seq // CHUNK          # i-chunks
    jpc = CHUNK // P            # j-tiles per chunk (4)
    njt = seq // P              # j-tiles
    nk = dim // P               # feature k-tiles
    nko = INNER // P            # inner k-tiles for the output projection

    nc = bacc.Bacc("TRN2", target_bir_lowering=False, debug=False,
                   enable_asserts=False, num_devices=n_cores)

    xT = nc.dram_tensor("xT", [dim, seq], BF16, kind="ExternalInput").ap()
    wq = nc.dram_tensor("wq", [dim, HS], BF16, kind="ExternalInput").ap()
    wk = nc.dram_tensor("wk", [dim, HS], BF16, kind="ExternalInput").ap()
    wv = nc.dram_tensor("wv", [dim, HS], BF16, kind="ExternalInput").ap()
    wo = nc.dram_tensor("wo", [INNER, HS], BF16, kind="ExternalInput").ap()
    mask_c = nc.dram_tensor("mask_c", [P, P], BF16, kind="ExternalInput").ap()
    out = nc.dram_tensor("out", [seq, HS], F32, kind="ExternalOutput").ap()

    with tile.TileContext(nc) as tc:
        with tc.tile_pool(name="sb", bufs=1) as sb, \
             tc.tile_pool(name="ps", bufs=1, space="PSUM") as ps, \
             tc.tile_pool(name="dram", bufs=1, space="DRAM") as dram:

            # ---- load inputs ----
            xt = [sb.tile([P, seq], BF16, tag=f"xt{k}", name=f"xt{k}")
                  for k in range(nk)]
            wq_sb = [sb.tile([P, HS], BF16, tag=f"wq{k}", name=f"wq{k}")
                     for k in range(nk)]
            wk_sb = [sb.tile([P, HS], BF16, tag=f"wk{k}", name=f"wk{k}")
                     for k in range(nk)]
            wv_sb = [sb.tile([P, HS], BF16, tag=f"wv{k}", name=f"wv{k}")
                     for k in range(nk)]
            wo_sb = [sb.tile([P, HS], BF16, tag=f"wo{k}", name=f"wo{k}")
                     for k in range(nko)]
            mask_sb = sb.tile([P, P], BF16, tag="mask", name="mask")

            for k in range(nk):
                nc.sync.dma_start(xt[k][:], xT[k * P:(k + 1) * P, :])
                nc.sync.dma_start(wq_sb[k][:], wq[k * P:(k + 1) * P, :])
                nc.sync.dma_start(wk_sb[k][:], wk[k * P:(k + 1) * P, :])
                nc.sync.dma_start(wv_sb[k][:], wv[k * P:(k + 1) * P, :])
            for k in range(nko):
                nc.sync.dma_start(wo_sb[k][:], wo[k * P:(k + 1) * P, :])
            nc.sync.dma_start(mask_sb[:], mask_c[:])

            # psum rotation for single-chain matmul groups
            _rr = [0]
            _rr_tags = [("s0", 1), ("s1", 1), ("misc", 2)]

            def rr_psum(shape, name):
                tag, bufs = _rr_tags[_rr[0] % 3]
                _rr[0] += 1
                return ps.tile(shape, F32, tag=tag, name=name, bufs=bufs)

            # ---- QKV projections ----
            # qt/kt: [128, seq] per head-pair, partitions = head dims
            # (pair p holds heads 2p, 2p+1).
            # v: [128, HPC*DHA] per j-tile, per-head blocks of 65 cols
            # (64 V dims + a ones column for the softmax denominators).
            qt_sb = [sb.tile([P, seq], BF16, tag=f"qt{p}", name=f"qt{p}")
                     for p in range(2)]
            kt_sb = [sb.tile([P, seq], BF16, tag=f"kt{p}", name=f"kt{p}")
                     for p in range(2)]
            v_sb = [sb.tile([P, HPC * DHA], BF16, tag=f"v{j}", name=f"v{j}")
                    for j in range(njt)]

            for pair in range(2):
                for w_sb, dst in ((wq_sb, qt_sb[pair]), (wk_sb, kt_sb[pair])):
                    for ch in range(nch):
                        pt = rr_psum([P, CHUNK], f"qk{pair}_{ch}")
                        for k in range(nk):
                            nc.tensor.matmul(
                                pt[:],
                                lhsT=w_sb[k][:, pair * P:(pair + 1) * P],
                                rhs=xt[k][:, ch * CHUNK:(ch + 1) * CHUNK],
                                start=(k == 0), stop=(k == nk - 1))
                        nc.vector.tensor_copy(
                            dst[:, ch * CHUNK:(ch + 1) * CHUNK], pt[:])
            for jt in range(njt):
                pt = rr_psum([P, HS], f"v{jt}")
                for k in range(nk):
                    nc.tensor.matmul(
                        pt[:], lhsT=xt[k][:, jt * P:(jt + 1) * P],
                        rhs=wv_sb[k][:],
                        start=(k == 0), stop=(k == nk - 1))
                # scatter head blocks into the augmented layout + ones col
                nc.vector.tensor_copy(
                    v_sb[jt].rearrange("p (h d) -> p h d", h=HPC)[:, :, 0:DH],
                    pt.rearrange("p (h d) -> p h d", h=HPC))
                nc.vector.memset(
                    v_sb[jt].rearrange("p (h d) -> p h d", h=HPC)[:, :, DH:DHA],
                    1.0)

            # ---- attention + allgather + output projection, per i-chunk ----
            ot_sb = [sb.tile([P, seq], BF16, tag=f"ot{p}", name=f"ot{p}")
                     for p in range(2)]

            for ci in range(nch):
                jt_end = jpc * (ci + 1)
                c0 = ci * CHUNK

                # per-head O^T psum [65, 512]: rows 0..63 = V^T @ expS^T,
                # row 64 = softmax denominators.
                ot_ps = [ps.tile([DHA, CHUNK], F32, tag=f"ot{h}",
                                 name=f"ot{ci}_{h}", bufs=1)
                         for h in range(HPC)]

                for jt in range(jt_end):
                    rel = max(0, (jt - jpc * ci)) * P
                    ncols = CHUNK - rel
                    diag = jt >= jpc * ci

                    s_ps = [ps.tile([P, CHUNK], F32, tag=f"s{h2}",
                                    name=f"s{ci}_{jt}_{h2}", bufs=1)
                            for h2 in range(2)]
                    es = [sb.tile([P, CHUNK], BF16, tag=f"es{h}",
                                  name=f"es{ci}_{jt}_{h}", bufs=2)
                          for h in range(HPC)]

                    for h in range(HPC):
                        pair, h2 = divmod(h, 2)
                        # S^T tile = K_h @ Q_h^T  (row-tiled pairs, K=64)
                        nc.tensor.matmul(
                            s_ps[h2][:, rel:CHUNK],
                            lhsT=kt_sb[pair][h2 * DH:(h2 + 1) * DH,
                                             jt * P:(jt + 1) * P],
                            rhs=qt_sb[pair][h2 * DH:(h2 + 1) * DH,
                                            c0 + rel:c0 + CHUNK],
                            start=True, stop=True,
                            tile_position=(h2 * DH, 0))
                        nc.scalar.activation(
                            es[h][:, rel:CHUNK], s_ps[h2][:, rel:CHUNK],
                            mybir.ActivationFunctionType.Exp)
                        if diag:
                            nc.vector.tensor_mul(
                                es[h][:, rel:rel + P],
                                es[h][:, rel:rel + P], mask_sb[:])
                    for h in range(HPC):
                        # O^T(+sums) accumulation: V_aug^T @ expS^T
                        nc.tensor.matmul(
                            ot_ps[h][:, rel:CHUNK],
                            lhsT=v_sb[jt][:, h * DHA:(h + 1) * DHA],
                            rhs=es[h][:, rel:CHUNK],
                            start=(jt == 0), stop=(jt == jt_end - 1))

                # normalize: ot[h] *= 1/sums[h] broadcast down the 64 dims
                for h in range(HPC):
                    pair, h2 = divmod(h, 2)
                    rs_sb = sb.tile([1, CHUNK], F32, tag=f"rs{h}",
                                    name=f"rs{ci}_{h}", bufs=2)
                    bc_sb = sb.tile([DH, CHUNK], F32, tag=f"bc{h}",
                                    name=f"bc{ci}_{h}", bufs=2)
                    nc.vector.reciprocal(rs_sb[:], ot_ps[h][DH:DHA, :])
                    nc.gpsimd.partition_broadcast(bc_sb[:], rs_sb[:],
                                                  channels=DH)
                    nc.vector.tensor_mul(
                        ot_sb[pair][h2 * DH:(h2 + 1) * DH, c0:c0 + CHUNK],
                        ot_ps[h][0:DH, :], bc_sb[:])

                # allgather this chunk's O^T across the 4-core group
                bounce_in = dram.tile([2 * P, CHUNK], BF16,
                                      tag="bin", name=f"bin{ci}", bufs=2)
                bounce_out = dram.tile([INNER, CHUNK], BF16,
                                       tag="bout", name=f"bout{ci}", bufs=2)
                for pair in range(2):
                    nc.sync.dma_start(bounce_in[pair * P:(pair + 1) * P, :],
                                      ot_sb[pair][:, c0:c0 + CHUNK])
                nc.gpsimd.collective_compute(
                    "AllGather", mybir.AluOpType.bypass,
                    replica_groups=groups,
                    ins=[bounce_in.opt()], outs=[bounce_out.opt()])

                # output projection for this chunk (column slice of Wo)
                ag_sb = [sb.tile([P, CHUNK], BF16, tag=f"ag{k}",
                                 name=f"ag{ci}_{k}", bufs=2)
                         for k in range(nko)]
                for k in range(nko):
                    nc.sync.dma_start(ag_sb[k][:],
                                      bounce_out[k * P:(k + 1) * P, :])
                for it in range(jpc):
                    op_ps = ps.tile([P, HS], F32, tag="misc",
                                    name=f"op{ci}_{it}", bufs=2)
                    for k in range(nko):
                        nc.tensor.matmul(
                            op_ps[:],
                            lhsT=ag_sb[k][:, it * P:(it + 1) * P],
                            rhs=wo_sb[k][:],
                            start=(k == 0), stop=(k == nko - 1))
                    o_sb = sb.tile([P, HS], F32, tag="osb",
                                   name=f"o{ci}_{it}", bufs=2)
                    nc.vector.tensor_copy(o_sb[:], op_ps[:])
                    nc.sync.dma_start(
                        out[c0 + it * P:c0 + (it + 1) * P, :], o_sb[:])

    if compile:
        nc.compile()
    return nc


def make_in_maps(x, Wq, Wk, Wv, Wo, n_cores=N_CORES):
    import ml_dtypes
    bf16 = ml_dtypes.bfloat16
    scale = np.float32(DH ** -0.5)
    # band mask for the diagonal j-tile of S^T [j,i]: keep j <= i
    mask_b = np.triu(np.ones((P, P), np.float32)).astype(bf16)
    in_maps = []
    for c in range(n_cores):
        b, r = divmod(c, 4)
        hs = r * HS
        in_maps.append({
            "xT": np.ascontiguousarray(x[b].T).astype(bf16),
            "wq": (Wq[:, hs:hs + HS] * scale).astype(bf16),
            "wk": np.ascontiguousarray(Wk[:, hs:hs + HS]).astype(bf16),
            "wv": np.ascontiguousarray(Wv[:, hs:hs + HS]).astype(bf16),
            "wo": np.ascontiguousarray(Wo[:, hs:hs + HS]).astype(bf16),
            "mask_c": mask_b,
        })
    return in_maps


def assemble_out(results, B, seq, n_cores=N_CORES):
    out = np.empty((B, seq, INNER), np.float32)
    for c in range(n_cores):
        b, r = divmod(c, 4)
        out[b][:, r * HS:(r + 1) * HS] = results[c]["out"]
    return out


_NC_CACHE = {}


def kernel(x, Wq, Wk, Wv, Wo):
    from concourse import bass_utils
    x = np.asarray(x, np.float32)
    B, seq, dim = x.shape
    key = (seq, dim)
    if key not in _NC_CACHE:
        _NC_CACHE[key] = build_nc(seq=seq, dim=dim)
    nc = _NC_CACHE[key]
    in_maps = make_in_maps(x, np.asarray(Wq, np.float32),
                           np.asarray(Wk, np.float32),
                           np.asarray(Wv, np.float32),
                           np.asarray(Wo, np.float32))
    res = bass_utils.run_bass_kernel_spmd(
        nc, in_maps, core_ids=list(range(N_CORES)))
    return assemble_out(res.results, B, seq)


# revision 6
# speedup vs baseline: 1.0312x; 1.0312x over previous
"""Distributed causal multi-head attention for one TRN2 chip (8 NeuronCores).

Sharding: batch (2) x head-groups (4 heads/core) -> 8 cores.
Core c handles batch c//4, heads [ (c%4)*4 , (c%4)*4+4 ).
Per core: QKV projections for its 4 heads, flash-style causal attention
with scores kept transposed (S^T = K @ Q^T) so the PV product needs no
transposes; V is augmented with a ones column so the softmax denominators
fall out of the same matmul (row 64 of each head's O^T psum).  Then an
AllGather of the attention output (pre-Wo, 4-core group = one batch) and
a column-sliced output projection.  Host assembles the 8 column/batch
shards.  Compute dtype bf16 (PSUM accumulation fp32), softmax in fp32.
"""

import sys
import numpy as np

sys.path.insert(0, "/opt/trn_rl_repo")

import concourse.bass as bass  # noqa: E402
import concourse.bacc as bacc  # noqa: E402
import concourse.tile as tile  # noqa: E402
import concourse.mybir as mybir  # noqa: E402

F32 = mybir.dt.float32
BF16 = mybir.dt.bfloat16

P = 128          # partition dim
CHUNK = 512      # i-chunk (matmul moving free dim)
DH = 64          # head dim
HPC = 4          # heads per core
HS = HPC * DH    # 256 per-core inner slice
DHA = DH + 1     # augmented head dim (ones column for softmax sums)
INNER = 1024     # total inner dim (16 heads x 64)
N_CORES = 8
GROUPS = [[0, 1, 2, 3], [4, 5, 6, 7]]


def build_nc(seq=2048, dim=1024, n_cores=N_CORES, groups=GROUPS, compile=True):
    """Build the SPMD Bass graph (identical on all cores)."""
    nch = seq // CHUNK          # i-chunks
    jpc = CHUNK // P            # j-tiles per chunk (4)
    njt = seq // P              # j-tiles
    nk = dim // P               # feature k-tiles
    nko = INNER // P            # inner k-tiles for the output projection

    nc = bacc.Bacc("TRN2", target_bir_lowering=False, debug=False,
                   enable_asserts=False, num_devices=n_cores)

    xT = nc.dram_tensor("xT", [dim, seq], BF16, kind="ExternalInput").ap()
    wq = nc.dram_tensor("wq", [dim, HS], BF16, kind="ExternalInput").ap()
    wk = nc.dram_tensor("wk", [dim, HS], BF16, kind="ExternalInput").ap()
    wv = nc.dram_tensor("wv", [dim, HS], BF16, kind="ExternalInput").ap()
    wo = nc.dram_tensor("wo", [INNER, HS], BF16, kind="ExternalInput").ap()
    mask_c = nc.dram_tensor("mask_c", [P, P], BF16, kind="ExternalInput").ap()
    out = nc.dram_tensor("out", [seq, HS], F32, kind="ExternalOutput").ap()

    with tile.TileContext(nc) as tc:
        with tc.tile_pool(name="sb", bufs=1) as sb, \
             tc.tile_pool(name="ps", bufs=1, space="PSUM") as ps, \
             tc.tile_pool(name="dram", bufs=1, space="DRAM") as dram:

            # ---- load inputs ----
            xt = [sb.tile([P, seq], BF16, tag=f"xt{k}", name=f"xt{k}")
                  for k in range(nk)]
            wq_sb = [sb.tile([P, HS], BF16, tag=f"wq{k}", name=f"wq{k}")
                     for k in range(nk)]
            wk_sb = [sb.tile([P, HS], BF16, tag=f"wk{k}", name=f"wk{k}")
                     for k in range(nk)]
            wv_sb = [sb.tile([P, HS], BF16, tag=f"wv{k}", name=f"wv{k}")
                     for k in range(nk)]
            wo_sb = [sb.tile([P, HS], BF16, tag=f"wo{k}", name=f"wo{k}")
                     for k in range(nko)]
            mask_sb = sb.tile([P, P], BF16, tag="mask", name="mask")

            for k in range(nk):
                nc.sync.dma_start(xt[k][:], xT[k * P:(k + 1) * P, :])
                nc.sync.dma_start(wq_sb[k][:], wq[k * P:(k + 1) * P, :])
                nc.sync.dma_start(wk_sb[k][:], wk[k * P:(k + 1) * P, :])
                nc.sync.dma_start(wv_sb[k][:], wv[k * P:(k + 1) * P, :])
            for k in range(nko):
                nc.sync.dma_start(wo_sb[k][:], wo[k * P:(k + 1) * P, :])
            nc.sync.dma_start(mask_sb[:], mask_c[:])

            # psum rotation for single-chain matmul groups
            _rr = [0]
            _rr_tags = [("s0", 1), ("s1", 1), ("misc", 2)]

            def rr_psum(shape, name):
                tag, bufs = _rr_tags[_rr[0] % 3]
                _rr[0] += 1
                return ps.tile(shape, F32, tag=tag, name=name, bufs=bufs)

            # ---- QKV projections ----
            # qt/kt: [128, seq] per head-pair, partitions = head dims
            # (pair p holds heads 2p, 2p+1).
            # v: [128, HPC*DHA] per j-tile, per-head blocks of 65 cols
            # (64 V dims + a ones column for the softmax denominators).
            qt_sb = [sb.tile([P, seq], BF16, tag=f"qt{p}", name=f"qt{p}")
                     for p in range(2)]
            kt_sb = [sb.tile([P, seq], BF16, tag=f"kt{p}", name=f"kt{p}")
                     for p in range(2)]
            v_sb = [sb.tile([P, HPC * DHA], BF16, tag=f"v{j}", name=f"v{j}")
                    for j in range(njt)]

            for pair in range(2):
                for w_sb, dst in ((wq_sb, qt_sb[pair]), (wk_sb, kt_sb[pair])):
                    for ch in range(nch):
                        pt = rr_psum([P, CHUNK], f"qk{pair}_{ch}")
                        for k in range(nk):
                            nc.tensor.matmul(
                                pt[:],
                                lhsT=w_sb[k][:, pair * P:(pair + 1) * P],
                                rhs=xt[k][:, ch * CHUNK:(ch + 1) * CHUNK],
                                start=(k == 0), stop=(k == nk - 1))
                        nc.vector.tensor_copy(
                            dst[:, ch * CHUNK:(ch + 1) * CHUNK], pt[:])
            for jt in range(njt):
                pt = rr_psum([P, HS], f"v{jt}")
                for k in range(nk):
                    nc.tensor.matmul(
                        pt[:], lhsT=xt[k][:, jt * P:(jt + 1) * P],
                        rhs=wv_sb[k][:],
                        start=(k == 0), stop=(k == nk - 1))
                # scatter head blocks into the augmented layout + ones col
                nc.vector.tensor_copy(
                    v_sb[jt].rearrange("p (h d) -> p h d", h=HPC)[:, :, 0:DH],
                    pt.rearrange("p (h d) -> p h d", h=HPC))
                nc.vector.memset(
                    v_sb[jt].rearrange("p (h d) -> p h d", h=HPC)[:, :, DH:DHA],
                    1.0)

            # ---- attention + allgather + output projection, per i-chunk ----
            ot_sb = [sb.tile([P, seq], BF16, tag=f"ot{p}", name=f"ot{p}")
                     for p in range(2)]

            def proj_chunk(ci, bounce_out):
                # output projection for chunk ci (column slice of Wo);
                # emitted one chunk late so the in-order PE queue never
                # stalls on the in-flight AllGather.
                c0 = ci * CHUNK
                ag_sb = [sb.tile([P, CHUNK], BF16, tag=f"ag{k}",
                                 name=f"ag{ci}_{k}", bufs=2)
                         for k in range(nko)]
                for k in range(nko):
                    nc.sync.dma_start(ag_sb[k][:],
                                      bounce_out[k * P:(k + 1) * P, :])
                for it in range(jpc):
                    op_ps = ps.tile([P, HS], F32, tag="misc",
                                    name=f"op{ci}_{it}", bufs=2)
                    for k in range(nko):
                        nc.tensor.matmul(
                            op_ps[:],
                            lhsT=ag_sb[k][:, it * P:(it + 1) * P],
                            rhs=wo_sb[k][:],
                            start=(k == 0), stop=(k == nko - 1))
                    o_sb = sb.tile([P, HS], F32, tag="osb",
                                   name=f"o{ci}_{it}", bufs=2)
                    nc.vector.tensor_copy(o_sb[:], op_ps[:])
                    nc.sync.dma_start(
                        out[c0 + it * P:c0 + (it + 1) * P, :], o_sb[:])

            pending_proj = []
            for ci in range(nch):
                jt_end = jpc * (ci + 1)
                c0 = ci * CHUNK

                # per-head O^T psum [65, 512]: rows 0..63 = V^T @ expS^T,
                # row 64 = softmax denominators.
                ot_ps = [ps.tile([DHA, CHUNK], F32, tag=f"ot{h}",
                                 name=f"ot{ci}_{h}", bufs=1)
                         for h in range(HPC)]

                for jt in range(jt_end):
                    rel = max(0, (jt - jpc * ci)) * P
                    ncols = CHUNK - rel
                    diag = jt >= jpc * ci

                    s_ps = [ps.tile([P, CHUNK], F32, tag=f"s{h2}",
                                    name=f"s{ci}_{jt}_{h2}", bufs=1)
                            for h2 in range(2)]
                    es = [sb.tile([P, CHUNK], BF16, tag=f"es{h}",
                                  name=f"es{ci}_{jt}_{h}", bufs=2)
                          for h in range(HPC)]

                    for h in range(HPC):
                        pair, h2 = divmod(h, 2)
                        # S^T tile = K_h @ Q_h^T  (row-tiled pairs, K=64)
                        nc.tensor.matmul(
                            s_ps[h2][:, rel:CHUNK],
                            lhsT=kt_sb[pair][h2 * DH:(h2 + 1) * DH,
                                             jt * P:(jt + 1) * P],
                            rhs=qt_sb[pair][h2 * DH:(h2 + 1) * DH,
                                            c0 + rel:c0 + CHUNK],
                            start=True, stop=True,
                            tile_position=(h2 * DH, 0))
                        nc.scalar.activation(
                            es[h][:, rel:CHUNK], s_ps[h2][:, rel:CHUNK],
                            mybir.ActivationFunctionType.Exp)
                        if diag:
                            nc.vector.tensor_mul(
                                es[h][:, rel:rel + P],
                                es[h][:, rel:rel + P], mask_sb[:])
                    for h in range(HPC):
                        # O^T(+sums) accumulation: V_aug^T @ expS^T
                        nc.tensor.matmul(
                            ot_ps[h][:, rel:CHUNK],
                            lhsT=v_sb[jt][:, h * DHA:(h + 1) * DHA],
                            rhs=es[h][:, rel:CHUNK],
                            start=(jt == 0), stop=(jt == jt_end - 1))

                # evacuate psum fast (frees banks for the next chunk), then
                # normalize off the PE critical path:
                # ot[h] *= 1/sums[h] broadcast down the 64 head dims
                for h in range(HPC):
                    pair, h2 = divmod(h, 2)
                    otr = sb.tile([DH, CHUNK], F32, tag=f"otr{h}",
                                  name=f"otr{ci}_{h}", bufs=2)
                    srow = sb.tile([1, CHUNK], F32, tag=f"sr{h}",
                                   name=f"sr{ci}_{h}", bufs=2)
                    nc.vector.tensor_copy(otr[:], ot_ps[h][0:DH, :])
                    nc.vector.tensor_copy(srow[:], ot_ps[h][DH:DHA, :])
                    rcp = sb.tile([1, CHUNK], F32, tag=f"rcp{h}",
                                  name=f"rcp{ci}_{h}", bufs=2)
                    nc.vector.reciprocal_approx_fast(rcp[:], srow[:])
                    bc_sb = sb.tile([DH, CHUNK], F32, tag=f"bc{h}",
                                    name=f"bc{ci}_{h}", bufs=2)
                    nc.gpsimd.partition_broadcast(bc_sb[:], rcp[:],
                                                  channels=DH)
                    nc.vector.tensor_mul(
                        ot_sb[pair][h2 * DH:(h2 + 1) * DH, c0:c0 + CHUNK],
                        otr[:], bc_sb[:])

                # allgather this chunk's O^T across the 4-core group
                bounce_in = dram.tile([2 * P, CHUNK], BF16,
                                      tag="bin", name=f"bin{ci}", bufs=2)
                bounce_out = dram.tile([INNER, CHUNK], BF16,
                                       tag="bout", name=f"bout{ci}", bufs=2)
                for pair in range(2):
                    nc.sync.dma_start(bounce_in[pair * P:(pair + 1) * P, :],
                                      ot_sb[pair][:, c0:c0 + CHUNK])
                nc.gpsimd.collective_compute(
                    "AllGather", mybir.AluOpType.bypass,
                    replica_groups=groups,
                    ins=[bounce_in.opt()], outs=[bounce_out.opt()])

                pending_proj.append((ci, bounce_out))
                if len(pending_proj) > 1:
                    proj_chunk(*pending_proj.pop(0))
            while pending_proj:
                proj_chunk(*pending_proj.pop(0))

    if compile:
        nc.compile()
    return nc


def make_in_maps(x, Wq, Wk, Wv, Wo, n_cores=N_CORES):
    import ml_dtypes
    bf16 = ml_dtypes.bfloat16
    scale = np.float32(DH ** -0.5)
    # band mask for the diagonal j-tile of S^T [j,i]: keep j <= i
    mask_b = np.triu(np.ones((P, P), np.float32)).astype(bf16)
    in_maps = []
    for c in range(n_cores):
        b, r = divmod(c, 4)
        hs = r * HS
        in_maps.append({
            "xT": np.ascontiguousarray(x[b].T).astype(bf16),
            "wq": (Wq[:, hs:hs + HS] * scale).astype(bf16),
            "wk": np.ascontiguousarray(Wk[:, hs:hs + HS]).astype(bf16),
            "wv": np.ascontiguousarray(Wv[:, hs:hs + HS]).astype(bf16),
            "wo": np.ascontiguousarray(Wo[:, hs:hs + HS]).astype(bf16),
            "mask_c": mask_b,
        })
    return in_maps


def assemble_out(results, B, seq, n_cores=N_CORES):
    out = np.empty((B, seq, INNER), np.float32)
    for c in range(n_cores):
        b, r = divmod(c, 4)
        out[b][:, r * HS:(r + 1) * HS] = results[c]["out"]
    return out


_NC_CACHE = {}


def kernel(x, Wq, Wk, Wv, Wo):
    from concourse import bass_utils
    x = np.asarray(x, np.float32)
    B, seq, dim = x.shape
    key = (seq, dim)
    if key not in _NC_CACHE:
        _NC_CACHE[key] = build_nc(seq=seq, dim=dim)
    nc = _NC_CACHE[key]
    in_maps = make_in_maps(x, np.asarray(Wq, np.float32),
                           np.asarray(Wk, np.float32),
                           np.asarray(Wv, np.float32),
                           np.asarray(Wo, np.float32))
    res = bass_utils.run_bass_kernel_spmd(
        nc, in_maps, core_ids=list(range(N_CORES)))
    return assemble_out(res.results, B, seq)


# revision 7
# speedup vs baseline: 1.2822x; 1.2434x over previous
"""Distributed causal multi-head attention for one TRN2 chip (8 NeuronCores).

Sharding: batch (2) x head-groups (4 heads/core) -> 8 cores.
Core c handles batch c//4, heads [ (c%4)*4 , (c%4)*4+4 ).
Per core: QKV projections for its 4 heads, flash-style causal attention
with scores kept transposed (S^T = K @ Q^T) so the PV product needs no
transposes; V is augmented with a ones column so the softmax denominators
fall out of the same matmul (row 64 of each head's O^T psum).  Then an
AllGather of the attention output (pre-Wo, 4-core group = one batch) and
a column-sliced output projection.  Host assembles the 8 column/batch
shards.  Compute dtype bf16 (PSUM accumulation fp32), softmax in fp32.

Scheduling: the attention phase is ScalarE(exp)-paced, so KT/V
projections for later chunks and the (AllGather-gated) output
projections are emitted as interleaved work items inside the attention
loops — the in-order PE queue then never stalls long enough for the
HAM clock gate to re-throttle, and collectives overlap compute.
"""

import sys
from collections import deque

import numpy as np

sys.path.insert(0, "/opt/trn_rl_repo")

import concourse.bass as bass  # noqa: E402
import concourse.bacc as bacc  # noqa: E402
import concourse.tile as tile  # noqa: E402
import concourse.mybir as mybir  # noqa: E402

F32 = mybir.dt.float32
BF16 = mybir.dt.bfloat16
ActFn = mybir.ActivationFunctionType

P = 128          # partition dim
CHUNK = 512      # i-chunk (matmul moving free dim, one psum bank of fp32)
DH = 64          # head dim
HPC = 4          # heads per core
HS = HPC * DH    # 256 per-core inner slice
DHA = DH + 1     # augmented head dim (ones column for softmax sums)
INNER = 1024     # total inner dim (16 heads x 64)
N_CORES = 8
GROUPS = [[0, 1, 2, 3], [4, 5, 6, 7]]


def build_nc(seq=2048, dim=1024, n_cores=N_CORES, groups=GROUPS, compile=True):
    """Build the SPMD Bass graph (identical on all cores)."""
    nch = seq // CHUNK          # i-chunks
    jpc = CHUNK // P            # j-tiles per chunk (4)
    njt = seq // P              # j-tiles
    nk = dim // P               # feature k-tiles
    nko = INNER // P            # inner k-tiles for the output projection

    nc = bacc.Bacc("TRN2", target_bir_lowering=False, debug=False,
                   enable_asserts=False, num_devices=n_cores)

    xT = nc.dram_tensor("xT", [dim, seq], BF16, kind="ExternalInput").ap()
    wq = nc.dram_tensor("wq", [dim, HS], BF16, kind="ExternalInput").ap()
    wk = nc.dram_tensor("wk", [dim, HS], BF16, kind="ExternalInput").ap()
    wv = nc.dram_tensor("wv", [dim, HS], BF16, kind="ExternalInput").ap()
    wo = nc.dram_tensor("wo", [INNER, HS], BF16, kind="ExternalInput").ap()
    mask_c = nc.dram_tensor("mask_c", [P, P], BF16, kind="ExternalInput").ap()
    out = nc.dram_tensor("out", [seq, HS], F32, kind="ExternalOutput").ap()

    with tile.TileContext(nc) as tc:
        with tc.tile_pool(name="sb", bufs=1) as sb, \
             tc.tile_pool(name="ps", bufs=1, space="PSUM") as ps, \
             tc.tile_pool(name="dram", bufs=1, space="DRAM") as dram:

            # ---- load inputs ----
            xt = [sb.tile([P, seq], BF16, tag=f"xt{k}", name=f"xt{k}")
                  for k in range(nk)]
            wq_sb = [sb.tile([P, HS], BF16, tag=f"wq{k}", name=f"wq{k}")
                     for k in range(nk)]
            wk_sb = [sb.tile([P, HS], BF16, tag=f"wk{k}", name=f"wk{k}")
                     for k in range(nk)]
            wv_sb = [sb.tile([P, HS], BF16, tag=f"wv{k}", name=f"wv{k}")
                     for k in range(nk)]
            wo_sb = [sb.tile([P, HS], BF16, tag=f"wo{k}", name=f"wo{k}")
                     for k in range(nko)]
            mask_sb = sb.tile([P, P], BF16, tag="mask", name="mask")

            for k in range(nk):
                nc.sync.dma_start(xt[k][:], xT[k * P:(k + 1) * P, :])
                nc.sync.dma_start(wq_sb[k][:], wq[k * P:(k + 1) * P, :])
                nc.sync.dma_start(wk_sb[k][:], wk[k * P:(k + 1) * P, :])
                nc.sync.dma_start(wv_sb[k][:], wv[k * P:(k + 1) * P, :])
            for k in range(nko):
                nc.sync.dma_start(wo_sb[k][:], wo[k * P:(k + 1) * P, :])
            nc.sync.dma_start(mask_sb[:], mask_c[:])

            # warm up the collectives firmware while QKV runs
            warm_in = dram.tile([P, 4], BF16, tag="warm_i", name="warm_i")
            warm_out = dram.tile([len(groups[0]) * P, 4], BF16,
                                 tag="warm_o", name="warm_o")
            nc.sync.dma_start(warm_in[:], mask_c[0:P, 0:4])
            nc.gpsimd.collective_compute(
                "AllGather", mybir.AluOpType.bypass, replica_groups=groups,
                ins=[warm_in.opt()], outs=[warm_out.opt()])

            # persistent QKV results
            qt_sb = [sb.tile([P, seq], BF16, tag=f"qt{p}", name=f"qt{p}")
                     for p in range(2)]
            kt_sb = [sb.tile([P, seq], BF16, tag=f"kt{p}", name=f"kt{p}")
                     for p in range(2)]
            v_sb = [sb.tile([P, HPC * DHA], BF16, tag=f"v{j}", name=f"v{j}")
                    for j in range(njt)]
            ot_sb = [sb.tile([P, seq], BF16, tag=f"ot{p}", name=f"ot{p}")
                     for p in range(2)]

            # ---- interleavable work items (each emits one psum group) ----
            def emit_kt(pair, ch):
                pt = ps.tile([P, CHUNK], F32, tag="misc",
                             name=f"ktps{pair}_{ch}", bufs=2)
                for k in range(nk):
                    nc.tensor.matmul(
                        pt[:], lhsT=wk_sb[k][:, pair * P:(pair + 1) * P],
                        rhs=xt[k][:, ch * CHUNK:(ch + 1) * CHUNK],
                        start=(k == 0), stop=(k == nk - 1))
                nc.scalar.activation(
                    kt_sb[pair][:, ch * CHUNK:(ch + 1) * CHUNK], pt[:],
                    ActFn.Copy)

            def emit_v(jt):
                pt = ps.tile([P, HS], F32, tag="misc",
                             name=f"vps{jt}", bufs=2)
                for k in range(nk):
                    nc.tensor.matmul(
                        pt[:], lhsT=xt[k][:, jt * P:(jt + 1) * P],
                        rhs=wv_sb[k][:],
                        start=(k == 0), stop=(k == nk - 1))
                nc.scalar.activation(
                    v_sb[jt].rearrange("p (h d) -> p h d", h=HPC)[:, :, 0:DH],
                    pt.rearrange("p (h d) -> p h d", h=HPC), ActFn.Copy)
                nc.vector.memset(
                    v_sb[jt].rearrange("p (h d) -> p h d", h=HPC)[:, :, DH:DHA],
                    1.0)

            def emit_proj(ci, it, ag_sb):
                c0 = ci * CHUNK
                op_ps = ps.tile([P, HS], F32, tag="misc",
                                name=f"op{ci}_{it}", bufs=2)
                for k in range(nko):
                    nc.tensor.matmul(
                        op_ps[:], lhsT=ag_sb[k][:, it * P:(it + 1) * P],
                        rhs=wo_sb[k][:],
                        start=(k == 0), stop=(k == nko - 1))
                o_sb = sb.tile([P, HS], F32, tag="osb",
                               name=f"o{ci}_{it}", bufs=2)
                nc.vector.tensor_copy(o_sb[:], op_ps[:])
                nc.sync.dma_start(
                    out[c0 + it * P:c0 + (it + 1) * P, :], o_sb[:])

            work_early = deque()   # KT/V for future chunks (not gated)
            work_late = deque()    # output projections (gated on AllGather)

            def pop_work(late_ok):
                if work_early:
                    work_early.popleft()()
                elif late_ok and work_late:
                    work_late.popleft()()

            # ---- upfront projections: all of Q, chunk-0 K, chunk-0 V ----
            for pair in range(2):
                for ch in range(nch):
                    pt = ps.tile([P, CHUNK], F32, tag="s2",
                                 name=f"qps{pair}_{ch}", bufs=2)
                    for k in range(nk):
                        nc.tensor.matmul(
                            pt[:], lhsT=wq_sb[k][:, pair * P:(pair + 1) * P],
                            rhs=xt[k][:, ch * CHUNK:(ch + 1) * CHUNK],
                            start=(k == 0), stop=(k == nk - 1))
                    nc.scalar.activation(
                        qt_sb[pair][:, ch * CHUNK:(ch + 1) * CHUNK], pt[:],
                        ActFn.Copy)
            for pair in range(2):
                emit_kt(pair, 0)
            for jt in range(jpc):
                emit_v(jt)

            # ---- attention chunks ----
            for ci in range(nch):
                jt_end = jpc * (ci + 1)
                c0 = ci * CHUNK

                if ci + 1 < nch:
                    for pair in range(2):
                        work_early.append(
                            lambda pair=pair, ch=ci + 1: emit_kt(pair, ch))
                    for jt in range(jpc * (ci + 1), jpc * (ci + 2)):
                        work_early.append(lambda jt=jt: emit_v(jt))

                for hpass in range(2):
                    # heads 2*hpass, 2*hpass+1  (== head pair `hpass`)
                    ot_ps = [ps.tile([DHA, CHUNK], F32, tag=f"ot{h2}",
                                     name=f"ot{ci}_{hpass}_{h2}", bufs=1)
                             for h2 in range(2)]
                    for jt in range(jt_end):
                        rel = max(0, (jt - jpc * ci)) * P
                        diag = jt >= jpc * ci

                        s2 = ps.tile([P, 2 * CHUNK], F32, tag="s2",
                                     name=f"s{ci}_{hpass}_{jt}", bufs=2)
                        es = sb.tile([P, 2 * CHUNK], BF16, tag="es",
                                     name=f"es{ci}_{hpass}_{jt}", bufs=3)

                        for h2 in range(2):
                            # S^T tile = K_h @ Q_h^T (row-tiled, K=64)
                            nc.tensor.matmul(
                                s2[:, h2 * CHUNK + rel:(h2 + 1) * CHUNK],
                                lhsT=kt_sb[hpass][h2 * DH:(h2 + 1) * DH,
                                                  jt * P:(jt + 1) * P],
                                rhs=qt_sb[hpass][h2 * DH:(h2 + 1) * DH,
                                                 c0 + rel:c0 + CHUNK],
                                start=True, stop=True,
                                tile_position=(h2 * DH, 0))
                        # one exp for both heads (both psum banks)
                        nc.scalar.activation(
                            es.rearrange("p (t c) -> p t c", t=2)[:, :, rel:],
                            s2.rearrange("p (t c) -> p t c", t=2)[:, :, rel:],
                            ActFn.Exp)
                        if diag:
                            for h2 in range(2):
                                nc.vector.tensor_mul(
                                    es[:, h2 * CHUNK + rel:
                                       h2 * CHUNK + rel + P],
                                    es[:, h2 * CHUNK + rel:
                                       h2 * CHUNK + rel + P],
                                    mask_sb[:])
                        for h2 in range(2):
                            h = 2 * hpass + h2
                            # O^T(+sums) accumulation: V_aug^T @ expS^T
                            nc.tensor.matmul(
                                ot_ps[h2][:, rel:CHUNK],
                                lhsT=v_sb[jt][:, h * DHA:(h + 1) * DHA],
                                rhs=es[:, h2 * CHUNK + rel:(h2 + 1) * CHUNK],
                                start=(jt == 0), stop=(jt == jt_end - 1))
                        pop_work(late_ok=(jt >= jt_end - 2))

                    # evacuate psum fast, normalize off the PE critical path
                    for h2 in range(2):
                        h = 2 * hpass + h2
                        otr = sb.tile([DH, CHUNK], F32, tag=f"otr{h2}",
                                      name=f"otr{ci}_{h}", bufs=2)
                        srow = sb.tile([1, CHUNK], F32, tag=f"sr{h2}",
                                       name=f"sr{ci}_{h}", bufs=2)
                        nc.vector.tensor_copy(otr[:], ot_ps[h2][0:DH, :])
                        nc.vector.tensor_copy(srow[:], ot_ps[h2][DH:DHA, :])
                        rcp = sb.tile([1, CHUNK], F32, tag=f"rcp{h2}",
                                      name=f"rcp{ci}_{h}", bufs=2)
                        nc.vector.reciprocal_approx_fast(rcp[:], srow[:])
                        bc_sb = sb.tile([DH, CHUNK], F32, tag=f"bc{h2}",
                                        name=f"bc{ci}_{h}", bufs=2)
                        nc.gpsimd.partition_broadcast(bc_sb[:], rcp[:],
                                                      channels=DH)
                        nc.vector.tensor_mul(
                            ot_sb[hpass][h2 * DH:(h2 + 1) * DH,
                                         c0:c0 + CHUNK],
                            otr[:], bc_sb[:])

                # allgather this chunk's O^T across the 4-core group
                bounce_in = dram.tile([2 * P, CHUNK], BF16,
                                      tag="bin", name=f"bin{ci}", bufs=2)
                bounce_out = dram.tile([INNER, CHUNK], BF16,
                                       tag="bout", name=f"bout{ci}", bufs=2)
                for pair in range(2):
                    nc.sync.dma_start(bounce_in[pair * P:(pair + 1) * P, :],
                                      ot_sb[pair][:, c0:c0 + CHUNK])
                nc.gpsimd.collective_compute(
                    "AllGather", mybir.AluOpType.bypass,
                    replica_groups=groups,
                    ins=[bounce_in.opt()], outs=[bounce_out.opt()])
                # gathered attT loads on the (mostly idle) gpsimd queue
                ag_sb = [sb.tile([P, CHUNK], BF16, tag=f"ag{k}",
                                 name=f"ag{ci}_{k}", bufs=2)
                         for k in range(nko)]
                for k in range(nko):
                    nc.gpsimd.dma_start(ag_sb[k][:],
                                        bounce_out[k * P:(k + 1) * P, :])
                for it in range(jpc):
                    work_late.append(
                        lambda ci=ci, it=it, ag=ag_sb: emit_proj(ci, it, ag))

            while work_early or work_late:
                pop_work(late_ok=True)

    if compile:
        nc.compile()
    return nc


def make_in_maps(x, Wq, Wk, Wv, Wo, n_cores=N_CORES):
    import ml_dtypes
    bf16 = ml_dtypes.bfloat16
    scale = np.float32(DH ** -0.5)
    # band mask for the diagonal j-tile of S^T [j,i]: keep j <= i
    mask_b = np.triu(np.ones((P, P), np.float32)).astype(bf16)
    in_maps = []
    for c in range(n_cores):
        b, r = divmod(c, 4)
        hs = r * HS
        in_maps.append({
            "xT": np.ascontiguousarray(x[b].T).astype(bf16),
            "wq": (Wq[:, hs:hs + HS] * scale).astype(bf16),
            "wk": np.ascontiguousarray(Wk[:, hs:hs + HS]).astype(bf16),
            "wv": np.ascontiguousarray(Wv[:, hs:hs + HS]).astype(bf16),
            "wo": np.ascontiguousarray(Wo[:, hs:hs + HS]).astype(bf16),
            "mask_c": mask_b,
        })
    return in_maps


def assemble_out(results, B, seq, n_cores=N_CORES):
    out = np.empty((B, seq, INNER), np.float32)
    for c in range(n_cores):
        b, r = divmod(c, 4)
        out[b][:, r * HS:(r + 1) * HS] = results[c]["out"]
    return out


_NC_CACHE = {}


def kernel(x, Wq, Wk, Wv, Wo):
    from concourse import bass_utils
    x = np.asarray(x, np.float32)
    B, seq, dim = x.shape
    key = (seq, dim)
    if key not in _NC_CACHE:
        _NC_CACHE[key] = build_nc(seq=seq, dim=dim)
    nc = _NC_CACHE[key]
    in_maps = make_in_maps(x, np.asarray(Wq, np.float32),
                           np.asarray(Wk, np.float32),
                           np.asarray(Wv, np.float32),
                           np.asarray(Wo, np.float32))
    res = bass_utils.run_bass_kernel_spmd(
        nc, in_maps, core_ids=list(range(N_CORES)))
    return assemble_out(res.results, B, seq)


# revision 10
# speedup vs baseline: 1.3347x; 1.0410x over previous
"""Distributed causal multi-head attention for one TRN2 chip (8 NeuronCores).

Sharding: batch (2) x head-groups (4 heads/core) -> 8 cores.
Core c handles batch c//4, heads [ (c%4)*4 , (c%4)*4+4 ).
Per core: QKV projections for its 4 heads, flash-style causal attention
with scores kept transposed (S^T = K @ Q^T) so the PV product needs no
transposes; V is augmented with a ones column so the softmax denominators
fall out of the same matmul (row 64 of each head's O^T psum).  Then an
AllGather of the attention output (pre-Wo, 4-core group = one batch) and
a column-sliced output projection.  Host assembles the 8 column/batch
shards.  Compute dtype bf16 (PSUM accumulation fp32), softmax in fp32.

Scheduling: the attention loop keeps the in-order PE queue saturated by
interleaving KT/V projections for later chunks and the AllGather-gated
output projections as queued work items (so the HAM clock gate never
re-throttles).  Chunk 0+1 share one AllGather (it completes under chunk
2's attention); the last chunk's AllGather is split per head-pair so the
first half overlaps pass B and the output projection can start on half
the k-tiles while the second half is still in flight.
"""

import sys
from collections import deque

import numpy as np

sys.path.insert(0, "/opt/trn_rl_repo")

import concourse.bass as bass  # noqa: E402
import concourse.bacc as bacc  # noqa: E402
import concourse.tile as tile  # noqa: E402
import concourse.mybir as mybir  # noqa: E402

F32 = mybir.dt.float32
BF16 = mybir.dt.bfloat16
ActFn = mybir.ActivationFunctionType

P = 128          # partition dim
CHUNK = 512      # i-chunk (matmul moving free dim, one psum bank of fp32)
DH = 64          # head dim
HPC = 4          # heads per core
HS = HPC * DH    # 256 per-core inner slice
DHA = DH + 1     # augmented head dim (ones column for softmax sums)
INNER = 1024     # total inner dim (16 heads x 64)
N_CORES = 8
GROUPS = [[0, 1, 2, 3], [4, 5, 6, 7]]


def build_nc(seq=2048, dim=1024, n_cores=N_CORES, groups=GROUPS, compile=True):
    """Build the SPMD Bass graph (identical on all cores)."""
    nch = seq // CHUNK          # i-chunks
    jpc = CHUNK // P            # j-tiles per chunk (4)
    njt = seq // P              # j-tiles
    nk = dim // P               # feature k-tiles
    nko = INNER // P            # inner k-tiles for the output projection
    grp = len(groups[0])        # replica group size (4)

    nc = bacc.Bacc("TRN2", target_bir_lowering=False, debug=False,
                   enable_asserts=False, num_devices=n_cores)

    xT = nc.dram_tensor("xT", [dim, seq], BF16, kind="ExternalInput").ap()
    wq = nc.dram_tensor("wq", [dim, HS], BF16, kind="ExternalInput").ap()
    wk = nc.dram_tensor("wk", [dim, HS], BF16, kind="ExternalInput").ap()
    wv = nc.dram_tensor("wv", [dim, HS], BF16, kind="ExternalInput").ap()
    wo = nc.dram_tensor("wo", [INNER, HS], BF16, kind="ExternalInput").ap()
    mask_c = nc.dram_tensor("mask_c", [P, P], BF16, kind="ExternalInput").ap()
    out = nc.dram_tensor("out", [seq, HS], F32, kind="ExternalOutput").ap()

    with tile.TileContext(nc) as tc:
        with tc.tile_pool(name="sb", bufs=1) as sb, \
             tc.tile_pool(name="ps", bufs=1, space="PSUM") as ps, \
             tc.tile_pool(name="dram", bufs=1, space="DRAM") as dram:

            # ---- load inputs ----
            xt = [sb.tile([P, seq], BF16, tag=f"xt{k}", name=f"xt{k}")
                  for k in range(nk)]
            wq_sb = [sb.tile([P, HS], BF16, tag=f"wq{k}", name=f"wq{k}")
                     for k in range(nk)]
            wk_sb = [sb.tile([P, HS], BF16, tag=f"wk{k}", name=f"wk{k}")
                     for k in range(nk)]
            wv_sb = [sb.tile([P, HS], BF16, tag=f"wv{k}", name=f"wv{k}")
                     for k in range(nk)]
            wo_sb = [sb.tile([P, HS], BF16, tag=f"wo{k}", name=f"wo{k}")
                     for k in range(nko)]
            mask_sb = sb.tile([P, P], BF16, tag="mask", name="mask")

            for k in range(nk):
                nc.sync.dma_start(xt[k][:], xT[k * P:(k + 1) * P, :])
                nc.sync.dma_start(wq_sb[k][:], wq[k * P:(k + 1) * P, :])
                nc.sync.dma_start(wk_sb[k][:], wk[k * P:(k + 1) * P, :])
                nc.sync.dma_start(wv_sb[k][:], wv[k * P:(k + 1) * P, :])
            for k in range(nko):
                nc.sync.dma_start(wo_sb[k][:], wo[k * P:(k + 1) * P, :])
            nc.sync.dma_start(mask_sb[:], mask_c[:])

            # warm up the collectives firmware while QKV runs
            warm_in = dram.tile([P, 4], BF16, tag="warm_i", name="warm_i")
            warm_out = dram.tile([grp * P, 4], BF16,
                                 tag="warm_o", name="warm_o")
            nc.sync.dma_start(warm_in[:], mask_c[0:P, 0:4])
            nc.gpsimd.collective_compute(
                "AllGather", mybir.AluOpType.bypass, replica_groups=groups,
                ins=[warm_in.opt()], outs=[warm_out.opt()])

            # persistent QKV results
            qt_sb = [sb.tile([P, seq], BF16, tag=f"qt{p}", name=f"qt{p}")
                     for p in range(2)]
            kt_sb = [sb.tile([P, seq], BF16, tag=f"kt{p}", name=f"kt{p}")
                     for p in range(2)]
            v_sb = [sb.tile([P, HPC * DHA], BF16, tag=f"v{j}", name=f"v{j}")
                    for j in range(njt)]
            ot_sb = [sb.tile([P, seq], BF16, tag=f"ot{p}", name=f"ot{p}")
                     for p in range(2)]

            # ---- interleavable work items (each emits one psum group) ----
            def emit_kt(pair, ch):
                pt = ps.tile([P, CHUNK], F32, tag="misc",
                             name=f"ktps{pair}_{ch}", bufs=2)
                for k in range(nk):
                    nc.tensor.matmul(
                        pt[:], lhsT=wk_sb[k][:, pair * P:(pair + 1) * P],
                        rhs=xt[k][:, ch * CHUNK:(ch + 1) * CHUNK],
                        start=(k == 0), stop=(k == nk - 1))
                nc.scalar.activation(
                    kt_sb[pair][:, ch * CHUNK:(ch + 1) * CHUNK], pt[:],
                    ActFn.Copy)

            def emit_v(jt):
                pt = ps.tile([P, HS], F32, tag="misc",
                             name=f"vps{jt}", bufs=2)
                for k in range(nk):
                    nc.tensor.matmul(
                        pt[:], lhsT=xt[k][:, jt * P:(jt + 1) * P],
                        rhs=wv_sb[k][:],
                        start=(k == 0), stop=(k == nk - 1))
                nc.scalar.activation(
                    v_sb[jt].rearrange("p (h d) -> p h d", h=HPC)[:, :, 0:DH],
                    pt.rearrange("p (h d) -> p h d", h=HPC), ActFn.Copy)
                nc.vector.memset(
                    v_sb[jt].rearrange("p (h d) -> p h d", h=HPC)[:, :, DH:DHA],
                    1.0)

            def emit_proj(ci, it, slices, korder):
                # one output i-tile: out[ci*CHUNK + it*128, :HS]
                # slices[k] = (ag_tile, col_offset) holding attT k-tile k
                c0 = ci * CHUNK
                op_ps = ps.tile([P, HS], F32, tag="misc",
                                name=f"op{ci}_{it}", bufs=2)
                for n, k in enumerate(korder):
                    ag_t, coff = slices[k]
                    nc.tensor.matmul(
                        op_ps[:],
                        lhsT=ag_t[:, coff + it * P:coff + (it + 1) * P],
                        rhs=wo_sb[k][:],
                        start=(n == 0), stop=(n == nko - 1))
                o_sb = sb.tile([P, HS], F32, tag="osb",
                               name=f"o{ci}_{it}", bufs=2)
                nc.vector.tensor_copy(o_sb[:], op_ps[:])
                nc.sync.dma_start(
                    out[c0 + it * P:c0 + (it + 1) * P, :], o_sb[:])

            work_early = deque()   # KT/V for future chunks (not gated)
            work_late = deque()    # output projections (gated on AllGather)

            def pop_work(late_ok):
                if work_early:
                    work_early.popleft()()
                elif late_ok and work_late:
                    work_late.popleft()()

            def push_proj(cis, ag, korder=None):
                korder = list(korder or range(nko))
                for n, ci in enumerate(cis):
                    slices = [(ag[k], n * CHUNK) for k in range(nko)]
                    for it in range(jpc):
                        work_late.append(
                            lambda ci=ci, it=it, s=slices, ko=korder:
                            emit_proj(ci, it, s, ko))

            def emit_ag(cis):
                # AllGather O^T for chunks `cis` -> ag tiles in SBUF
                w = len(cis) * CHUNK
                c0 = cis[0] * CHUNK
                bounce_in = dram.tile([2 * P, w], BF16, tag=f"bin{w}",
                                      name=f"bin{cis[0]}", bufs=2)
                bounce_out = dram.tile([grp * 2 * P, w], BF16, tag=f"bout{w}",
                                       name=f"bout{cis[0]}", bufs=2)
                for pair in range(2):
                    nc.sync.dma_start(bounce_in[pair * P:(pair + 1) * P, :],
                                      ot_sb[pair][:, c0:c0 + w])
                nc.gpsimd.collective_compute(
                    "AllGather", mybir.AluOpType.bypass,
                    replica_groups=groups,
                    ins=[bounce_in.opt()], outs=[bounce_out.opt()])
                ag = [sb.tile([P, w], BF16, tag=f"ag{k}",
                              name=f"ag{cis[0]}_{k}", bufs=2)
                      for k in range(nko)]
                for k in range(nko):
                    nc.sync.dma_start(ag[k][:],
                                      bounce_out[k * P:(k + 1) * P, :])
                return ag

            def emit_ag_pair(ci, pair):
                # half AllGather (one head pair) of the chunk `ci`
                c0 = ci * CHUNK
                bounce_in = dram.tile([P, CHUNK], BF16, tag=f"binh{pair}",
                                      name=f"binh{ci}_{pair}", bufs=1)
                bounce_out = dram.tile([grp * P, CHUNK], BF16,
                                       tag=f"bouth{pair}",
                                       name=f"bouth{ci}_{pair}", bufs=1)
                nc.sync.dma_start(bounce_in[:], ot_sb[pair][:, c0:c0 + CHUNK])
                nc.gpsimd.collective_compute(
                    "AllGather", mybir.AluOpType.bypass,
                    replica_groups=groups,
                    ins=[bounce_in.opt()], outs=[bounce_out.opt()])
                return bounce_out

            # ---- upfront projections: all of Q, chunk-0 K, chunk-0 V ----
            # Q is k-outer / weight-stationary so it pipelines with the xT
            # DMAs; two chunks share one 2-bank psum tile.
            for pair in range(2):
                for chh in range(0, nch, 2):
                    wch = min(2, nch - chh)
                    pt = ps.tile([P, wch * CHUNK], F32, tag="s2",
                                 name=f"qps{pair}_{chh}", bufs=2)
                    for k in range(nk):
                        for c in range(wch):
                            nc.tensor.matmul(
                                pt[:, c * CHUNK:(c + 1) * CHUNK],
                                lhsT=wq_sb[k][:, pair * P:(pair + 1) * P],
                                rhs=xt[k][:, (chh + c) * CHUNK:
                                           (chh + c + 1) * CHUNK],
                                start=(k == 0), stop=(k == nk - 1))
                    nc.scalar.activation(
                        qt_sb[pair][:, chh * CHUNK:(chh + wch) * CHUNK],
                        pt[:], ActFn.Copy)
            for pair in range(2):
                emit_kt(pair, 0)
            for jt in range(jpc):
                emit_v(jt)

            # ---- attention chunks ----
            last_parts = {}
            for ci in range(nch):
                jt_end = jpc * (ci + 1)
                c0 = ci * CHUNK
                last = ci == nch - 1

                if ci + 1 < nch:
                    for pair in range(2):
                        work_early.append(
                            lambda pair=pair, ch=ci + 1: emit_kt(pair, ch))
                    for jt in range(jpc * (ci + 1), jpc * (ci + 2)):
                        work_early.append(lambda jt=jt: emit_v(jt))

                for hpass in range(2):
                    # heads 2*hpass, 2*hpass+1  (== head pair `hpass`)
                    ot_ps = [ps.tile([DHA, CHUNK], F32, tag=f"ot{h2}",
                                     name=f"ot{ci}_{hpass}_{h2}", bufs=1)
                             for h2 in range(2)]
                    for jt in range(jt_end):
                        rel = max(0, (jt - jpc * ci)) * P
                        diag = jt >= jpc * ci

                        s2 = ps.tile([P, 2 * CHUNK], F32, tag="s2",
                                     name=f"s{ci}_{hpass}_{jt}", bufs=2)
                        es = sb.tile([P, 2 * CHUNK], BF16, tag="es",
                                     name=f"es{ci}_{hpass}_{jt}", bufs=3)

                        for h2 in range(2):
                            # S^T tile = K_h @ Q_h^T (row-tiled, K=64)
                            nc.tensor.matmul(
                                s2[:, h2 * CHUNK + rel:(h2 + 1) * CHUNK],
                                lhsT=kt_sb[hpass][h2 * DH:(h2 + 1) * DH,
                                                  jt * P:(jt + 1) * P],
                                rhs=qt_sb[hpass][h2 * DH:(h2 + 1) * DH,
                                                 c0 + rel:c0 + CHUNK],
                                start=True, stop=True,
                                tile_position=(h2 * DH, 0))
                        # one exp for both heads (both psum banks)
                        nc.scalar.activation(
                            es.rearrange("p (t c) -> p t c", t=2)[:, :, rel:],
                            s2.rearrange("p (t c) -> p t c", t=2)[:, :, rel:],
                            ActFn.Exp)
                        if diag:
                            for h2 in range(2):
                                nc.vector.tensor_mul(
                                    es[:, h2 * CHUNK + rel:
                                       h2 * CHUNK + rel + P],
                                    es[:, h2 * CHUNK + rel:
                                       h2 * CHUNK + rel + P],
                                    mask_sb[:])
                        for h2 in range(2):
                            h = 2 * hpass + h2
                            # O^T(+sums) accumulation: V_aug^T @ expS^T
                            nc.tensor.matmul(
                                ot_ps[h2][:, rel:CHUNK],
                                lhsT=v_sb[jt][:, h * DHA:(h + 1) * DHA],
                                rhs=es[:, h2 * CHUNK + rel:(h2 + 1) * CHUNK],
                                start=(jt == 0), stop=(jt == jt_end - 1))
                        pop_work(late_ok=(jt >= jt_end - 2))

                    # evacuate psum fast, normalize off the PE critical path
                    for h2 in range(2):
                        h = 2 * hpass + h2
                        otr = sb.tile([DH, CHUNK], F32, tag=f"otr{h2}",
                                      name=f"otr{ci}_{h}", bufs=2)
                        srow = sb.tile([1, CHUNK], F32, tag=f"sr{h2}",
                                       name=f"sr{ci}_{h}", bufs=2)
                        nc.vector.tensor_copy(otr[:], ot_ps[h2][0:DH, :])
                        nc.vector.tensor_copy(srow[:], ot_ps[h2][DH:DHA, :])
                        rcp = sb.tile([1, CHUNK], F32, tag=f"rcp{h2}",
                                      name=f"rcp{ci}_{h}", bufs=2)
                        nc.vector.reciprocal_approx_fast(rcp[:], srow[:])
                        bc_sb = sb.tile([DH, CHUNK], F32, tag=f"bc{h2}",
                                        name=f"bc{ci}_{h}", bufs=2)
                        nc.gpsimd.partition_broadcast(bc_sb[:], rcp[:],
                                                      channels=DH)
                        nc.vector.tensor_mul(
                            ot_sb[hpass][h2 * DH:(h2 + 1) * DH,
                                         c0:c0 + CHUNK],
                            otr[:], bc_sb[:])

                    if last:
                        # split AllGather: this pair's half goes out now
                        last_parts[hpass] = emit_ag_pair(ci, hpass)

                # AllGathers + gated projections
                if last:
                    # load both halves (emitted after all normalizes so the
                    # sync queue is past every producer), evens then odds
                    ag3 = {}
                    for pair in range(2):
                        bo = last_parts[pair]
                        for r in range(grp):
                            k = 2 * r + pair
                            t = sb.tile([P, CHUNK], BF16, tag=f"ag{k}",
                                        name=f"agL{k}", bufs=2)
                            nc.sync.dma_start(t[:],
                                              bo[r * P:(r + 1) * P, :])
                            ag3[k] = t
                    slices = [(ag3[k], 0) for k in range(nko)]
                    korder = [k for k in range(nko) if k % 2 == 0] + \
                             [k for k in range(nko) if k % 2 == 1]
                    for it in range(jpc):
                        work_late.append(
                            lambda ci=ci, it=it, s=slices, ko=korder:
                            emit_proj(ci, it, s, ko))
                elif ci == 1 or (ci == 0 and nch == 2):
                    cis = [0, 1] if ci == 1 else [0]
                    push_proj(cis, emit_ag(cis))
                elif ci >= 2:
                    push_proj([ci], emit_ag([ci]))

            while work_early or work_late:
                pop_work(late_ok=True)

    if compile:
        nc.compile()
    return nc


def make_in_maps(x, Wq, Wk, Wv, Wo, n_cores=N_CORES):
    import ml_dtypes
    bf16 = ml_dtypes.bfloat16
    scale = np.float32(DH ** -0.5)
    # band mask for the diagonal j-tile of S^T [j,i]: keep j <= i
    mask_b = np.triu(np.ones((P, P), np.float32)).astype(bf16)
    in_maps = []
    for c in range(n_cores):
        b, r = divmod(c, 4)
        hs = r * HS
        in_maps.append({
            "xT": np.ascontiguousarray(x[b].T).astype(bf16),
            "wq": (Wq[:, hs:hs + HS] * scale).astype(bf16),
            "wk": np.ascontiguousarray(Wk[:, hs:hs + HS]).astype(bf16),
            "wv": np.ascontiguousarray(Wv[:, hs:hs + HS]).astype(bf16),
            "wo": np.ascontiguousarray(Wo[:, hs:hs + HS]).astype(bf16),
            "mask_c": mask_b,
        })
    return in_maps


def assemble_out(results, B, seq, n_cores=N_CORES):
    out = np.empty((B, seq, INNER), np.float32)
    for c in range(n_cores):
        b, r = divmod(c, 4)
        out[b][:, r * HS:(r + 1) * HS] = results[c]["out"]
    return out


_NC_CACHE = {}


def kernel(x, Wq, Wk, Wv, Wo):
    from concourse import bass_utils
    x = np.asarray(x, np.float32)
    B, seq, dim = x.shape
    key = (seq, dim)
    if key not in _NC_CACHE:
        _NC_CACHE[key] = build_nc(seq=seq, dim=dim)
    nc = _NC_CACHE[key]
    in_maps = make_in_maps(x, np.asarray(Wq, np.float32),
                           np.asarray(Wk, np.float32),
                           np.asarray(Wv, np.float32),
                           np.asarray(Wo, np.float32))
    res = bass_utils.run_bass_kernel_spmd(
        nc, in_maps, core_ids=list(range(N_CORES)))
    return assemble_out(res.results, B, seq)


# revision 11
# speedup vs baseline: 1.3418x; 1.0053x over previous
"""Distributed causal multi-head attention for one TRN2 chip (8 NeuronCores).

Sharding: batch (2) x head-groups (4 heads/core) -> 8 cores.
Core c handles batch c//4, heads [ (c%4)*4 , (c%4)*4+4 ).
Per core: QKV projections for its 4 heads, flash-style causal attention
with scores kept transposed (S^T = K @ Q^T) so the PV product needs no
transposes; V is augmented with a ones column so the softmax denominators
fall out of the same matmul (row 64 of each head's O^T psum).  Then an
AllGather of the attention output (pre-Wo, 4-core group = one batch) and
a column-sliced output projection.  Host assembles the 8 column/batch
shards.  Compute dtype bf16 (PSUM accumulation fp32), softmax in fp32.

Scheduling: the attention loop keeps the in-order PE queue saturated by
interleaving KT/V projections for later chunks and the AllGather-gated
output projections as queued work items (so the HAM clock gate never
re-throttles).  Chunk 0+1 share one AllGather (it completes under chunk
2's attention); the last chunk's AllGather is split per head-pair so the
first half overlaps pass B and the output projection can start on half
the k-tiles while the second half is still in flight.
"""

import sys
from collections import deque

import numpy as np

sys.path.insert(0, "/opt/trn_rl_repo")

import concourse.bass as bass  # noqa: E402
import concourse.bacc as bacc  # noqa: E402
import concourse.tile as tile  # noqa: E402
import concourse.mybir as mybir  # noqa: E402

F32 = mybir.dt.float32
BF16 = mybir.dt.bfloat16
ActFn = mybir.ActivationFunctionType

P = 128          # partition dim
CHUNK = 512      # i-chunk (matmul moving free dim, one psum bank of fp32)
DH = 64          # head dim
HPC = 4          # heads per core
HS = HPC * DH    # 256 per-core inner slice
DHA = DH + 1     # augmented head dim (ones column for softmax sums)
INNER = 1024     # total inner dim (16 heads x 64)
N_CORES = 8
GROUPS = [[0, 1, 2, 3], [4, 5, 6, 7]]


def build_nc(seq=2048, dim=1024, n_cores=N_CORES, groups=GROUPS, compile=True):
    """Build the SPMD Bass graph (identical on all cores)."""
    nch = seq // CHUNK          # i-chunks
    jpc = CHUNK // P            # j-tiles per chunk (4)
    njt = seq // P              # j-tiles
    nk = dim // P               # feature k-tiles
    nko = INNER // P            # inner k-tiles for the output projection
    grp = len(groups[0])        # replica group size (4)

    nc = bacc.Bacc("TRN2", target_bir_lowering=False, debug=False,
                   enable_asserts=False, num_devices=n_cores)

    xT = nc.dram_tensor("xT", [dim, seq], BF16, kind="ExternalInput").ap()
    wq = nc.dram_tensor("wq", [dim, HS], BF16, kind="ExternalInput").ap()
    wk = nc.dram_tensor("wk", [dim, HS], BF16, kind="ExternalInput").ap()
    wv = nc.dram_tensor("wv", [dim, HS], BF16, kind="ExternalInput").ap()
    wo = nc.dram_tensor("wo", [INNER, HS], BF16, kind="ExternalInput").ap()
    mask_c = nc.dram_tensor("mask_c", [P, P], BF16, kind="ExternalInput").ap()
    out = nc.dram_tensor("out", [seq, HS], F32, kind="ExternalOutput").ap()

    with tile.TileContext(nc) as tc:
        with tc.tile_pool(name="sb", bufs=1) as sb, \
             tc.tile_pool(name="ps", bufs=1, space="PSUM") as ps, \
             tc.tile_pool(name="dram", bufs=1, space="DRAM") as dram:

            # ---- load inputs ----
            xt = [sb.tile([P, seq], BF16, tag=f"xt{k}", name=f"xt{k}")
                  for k in range(nk)]
            wq_sb = [sb.tile([P, HS], BF16, tag=f"wq{k}", name=f"wq{k}")
                     for k in range(nk)]
            wk_sb = [sb.tile([P, HS], BF16, tag=f"wk{k}", name=f"wk{k}")
                     for k in range(nk)]
            wv_sb = [sb.tile([P, HS], BF16, tag=f"wv{k}", name=f"wv{k}")
                     for k in range(nk)]
            wo_sb = [sb.tile([P, HS], BF16, tag=f"wo{k}", name=f"wo{k}")
                     for k in range(nko)]
            mask_sb = sb.tile([P, P], BF16, tag="mask", name="mask")

            for k in range(nk):
                nc.sync.dma_start(xt[k][:], xT[k * P:(k + 1) * P, :])
                nc.sync.dma_start(wq_sb[k][:], wq[k * P:(k + 1) * P, :])
                nc.sync.dma_start(wk_sb[k][:], wk[k * P:(k + 1) * P, :])
                nc.sync.dma_start(wv_sb[k][:], wv[k * P:(k + 1) * P, :])
            for k in range(nko):
                nc.sync.dma_start(wo_sb[k][:], wo[k * P:(k + 1) * P, :])
            nc.sync.dma_start(mask_sb[:], mask_c[:])

            # warm up the collectives firmware while QKV runs
            warm_in = dram.tile([P, 4], BF16, tag="warm_i", name="warm_i")
            warm_out = dram.tile([grp * P, 4], BF16,
                                 tag="warm_o", name="warm_o")
            nc.sync.dma_start(warm_in[:], mask_c[0:P, 0:4])
            nc.gpsimd.collective_compute(
                "AllGather", mybir.AluOpType.bypass, replica_groups=groups,
                ins=[warm_in.opt()], outs=[warm_out.opt()])

            # persistent QKV results
            qt_sb = [sb.tile([P, seq], BF16, tag=f"qt{p}", name=f"qt{p}")
                     for p in range(2)]
            kt_sb = [sb.tile([P, seq], BF16, tag=f"kt{p}", name=f"kt{p}")
                     for p in range(2)]
            v_sb = [sb.tile([P, HPC * DHA], BF16, tag=f"v{j}", name=f"v{j}")
                    for j in range(njt)]
            ot_sb = [sb.tile([P, seq], BF16, tag=f"ot{p}", name=f"ot{p}")
                     for p in range(2)]

            # ---- interleavable work items (each emits one psum group) ----
            def emit_kt(pair, ch):
                pt = ps.tile([P, CHUNK], F32, tag="misc",
                             name=f"ktps{pair}_{ch}", bufs=2)
                for k in range(nk):
                    nc.tensor.matmul(
                        pt[:], lhsT=wk_sb[k][:, pair * P:(pair + 1) * P],
                        rhs=xt[k][:, ch * CHUNK:(ch + 1) * CHUNK],
                        start=(k == 0), stop=(k == nk - 1))
                nc.scalar.activation(
                    kt_sb[pair][:, ch * CHUNK:(ch + 1) * CHUNK], pt[:],
                    ActFn.Copy)

            def emit_v(jt):
                pt = ps.tile([P, HS], F32, tag="misc",
                             name=f"vps{jt}", bufs=2)
                for k in range(nk):
                    nc.tensor.matmul(
                        pt[:], lhsT=xt[k][:, jt * P:(jt + 1) * P],
                        rhs=wv_sb[k][:],
                        start=(k == 0), stop=(k == nk - 1))
                nc.scalar.activation(
                    v_sb[jt].rearrange("p (h d) -> p h d", h=HPC)[:, :, 0:DH],
                    pt.rearrange("p (h d) -> p h d", h=HPC), ActFn.Copy)
                nc.vector.memset(
                    v_sb[jt].rearrange("p (h d) -> p h d", h=HPC)[:, :, DH:DHA],
                    1.0)

            def emit_proj(ci, it, slices, korder):
                # one output i-tile: out[ci*CHUNK + it*128, :HS]
                # slices[k] = (ag_tile, col_offset) holding attT k-tile k
                c0 = ci * CHUNK
                op_ps = ps.tile([P, HS], F32, tag="misc",
                                name=f"op{ci}_{it}", bufs=2)
                for n, k in enumerate(korder):
                    ag_t, coff = slices[k]
                    nc.tensor.matmul(
                        op_ps[:],
                        lhsT=ag_t[:, coff + it * P:coff + (it + 1) * P],
                        rhs=wo_sb[k][:],
                        start=(n == 0), stop=(n == nko - 1))
                o_sb = sb.tile([P, HS], F32, tag="osb",
                               name=f"o{ci}_{it}", bufs=2)
                nc.vector.tensor_copy(o_sb[:], op_ps[:])
                nc.sync.dma_start(
                    out[c0 + it * P:c0 + (it + 1) * P, :], o_sb[:])

            work_early = deque()   # KT/V for future chunks (not gated)
            work_late = deque()    # output projections (gated on AllGather)

            def pop_work(late_ok):
                if work_early:
                    work_early.popleft()()
                elif late_ok and work_late:
                    work_late.popleft()()

            def emit_ag_pair(ci, pair):
                # half AllGather (one head pair) of the chunk `ci` — fired
                # right after that pair's normalize, so pair A overlaps the
                # second attention pass and both stay small (cheap on CC)
                c0 = ci * CHUNK
                bounce_in = dram.tile([P, CHUNK], BF16, tag=f"binh{pair}",
                                      name=f"binh{ci}_{pair}", bufs=2)
                bounce_out = dram.tile([grp * P, CHUNK], BF16,
                                       tag=f"bouth{pair}",
                                       name=f"bouth{ci}_{pair}", bufs=2)
                nc.sync.dma_start(bounce_in[:], ot_sb[pair][:, c0:c0 + CHUNK])
                nc.gpsimd.collective_compute(
                    "AllGather", mybir.AluOpType.bypass,
                    replica_groups=groups,
                    ins=[bounce_in.opt()], outs=[bounce_out.opt()])
                return bounce_out

            # ---- upfront projections: all of Q, chunk-0 K, chunk-0 V ----
            # Q is k-outer / weight-stationary so it pipelines with the xT
            # DMAs; two chunks share one 2-bank psum tile.
            for pair in range(2):
                for chh in range(0, nch, 2):
                    wch = min(2, nch - chh)
                    pt = ps.tile([P, wch * CHUNK], F32, tag="s2",
                                 name=f"qps{pair}_{chh}", bufs=2)
                    for k in range(nk):
                        for c in range(wch):
                            nc.tensor.matmul(
                                pt[:, c * CHUNK:(c + 1) * CHUNK],
                                lhsT=wq_sb[k][:, pair * P:(pair + 1) * P],
                                rhs=xt[k][:, (chh + c) * CHUNK:
                                           (chh + c + 1) * CHUNK],
                                start=(k == 0), stop=(k == nk - 1))
                    nc.scalar.activation(
                        qt_sb[pair][:, chh * CHUNK:(chh + wch) * CHUNK],
                        pt[:], ActFn.Copy)
            for pair in range(2):
                emit_kt(pair, 0)
            for jt in range(jpc):
                emit_v(jt)

            # ---- attention chunks ----
            last_parts = {}
            for ci in range(nch):
                jt_end = jpc * (ci + 1)
                c0 = ci * CHUNK
                last = ci == nch - 1

                if ci + 1 < nch:
                    for pair in range(2):
                        work_early.append(
                            lambda pair=pair, ch=ci + 1: emit_kt(pair, ch))
                    for jt in range(jpc * (ci + 1), jpc * (ci + 2)):
                        work_early.append(lambda jt=jt: emit_v(jt))

                for hpass in range(2):
                    # heads 2*hpass, 2*hpass+1  (== head pair `hpass`)
                    ot_ps = [ps.tile([DHA, CHUNK], F32, tag=f"ot{h2}",
                                     name=f"ot{ci}_{hpass}_{h2}", bufs=1)
                             for h2 in range(2)]
                    for jt in range(jt_end):
                        rel = max(0, (jt - jpc * ci)) * P
                        diag = jt >= jpc * ci

                        s2 = ps.tile([P, 2 * CHUNK], F32, tag="s2",
                                     name=f"s{ci}_{hpass}_{jt}", bufs=2)
                        es = sb.tile([P, 2 * CHUNK], BF16, tag="es",
                                     name=f"es{ci}_{hpass}_{jt}", bufs=3)

                        for h2 in range(2):
                            # S^T tile = K_h @ Q_h^T (row-tiled, K=64)
                            nc.tensor.matmul(
                                s2[:, h2 * CHUNK + rel:(h2 + 1) * CHUNK],
                                lhsT=kt_sb[hpass][h2 * DH:(h2 + 1) * DH,
                                                  jt * P:(jt + 1) * P],
                                rhs=qt_sb[hpass][h2 * DH:(h2 + 1) * DH,
                                                 c0 + rel:c0 + CHUNK],
                                start=True, stop=True,
                                tile_position=(h2 * DH, 0))
                        # one exp for both heads (both psum banks)
                        nc.scalar.activation(
                            es.rearrange("p (t c) -> p t c", t=2)[:, :, rel:],
                            s2.rearrange("p (t c) -> p t c", t=2)[:, :, rel:],
                            ActFn.Exp)
                        if diag:
                            for h2 in range(2):
                                nc.vector.tensor_mul(
                                    es[:, h2 * CHUNK + rel:
                                       h2 * CHUNK + rel + P],
                                    es[:, h2 * CHUNK + rel:
                                       h2 * CHUNK + rel + P],
                                    mask_sb[:])
                        for h2 in range(2):
                            h = 2 * hpass + h2
                            # O^T(+sums) accumulation: V_aug^T @ expS^T
                            nc.tensor.matmul(
                                ot_ps[h2][:, rel:CHUNK],
                                lhsT=v_sb[jt][:, h * DHA:(h + 1) * DHA],
                                rhs=es[:, h2 * CHUNK + rel:(h2 + 1) * CHUNK],
                                start=(jt == 0), stop=(jt == jt_end - 1))
                        pop_work(late_ok=(jt >= jt_end - 2))

                    # evacuate psum fast, normalize off the PE critical path
                    for h2 in range(2):
                        h = 2 * hpass + h2
                        otr = sb.tile([DH, CHUNK], F32, tag=f"otr{h2}",
                                      name=f"otr{ci}_{h}", bufs=2)
                        srow = sb.tile([1, CHUNK], F32, tag=f"sr{h2}",
                                       name=f"sr{ci}_{h}", bufs=2)
                        nc.vector.tensor_copy(otr[:], ot_ps[h2][0:DH, :])
                        nc.vector.tensor_copy(srow[:], ot_ps[h2][DH:DHA, :])
                        rcp = sb.tile([1, CHUNK], F32, tag=f"rcp{h2}",
                                      name=f"rcp{ci}_{h}", bufs=2)
                        nc.vector.reciprocal_approx_fast(rcp[:], srow[:])
                        bc_sb = sb.tile([DH, CHUNK], F32, tag=f"bc{h2}",
                                        name=f"bc{ci}_{h}", bufs=2)
                        nc.gpsimd.partition_broadcast(bc_sb[:], rcp[:],
                                                      channels=DH)
                        nc.vector.tensor_mul(
                            ot_sb[hpass][h2 * DH:(h2 + 1) * DH,
                                         c0:c0 + CHUNK],
                            otr[:], bc_sb[:])

                    # this pair's half of the chunk goes out now
                    last_parts[hpass] = emit_ag_pair(ci, hpass)

                # gathered attT loads (after both bounce writes, so the sync
                # queue is past every producer), evens then odds; then the
                # gated output projections
                agt = {}
                for pair in range(2):
                    bo = last_parts[pair]
                    for r in range(grp):
                        k = 2 * r + pair
                        t = sb.tile([P, CHUNK], BF16, tag=f"ag{k}",
                                    name=f"ag{ci}_{k}", bufs=2)
                        nc.sync.dma_start(t[:], bo[r * P:(r + 1) * P, :])
                        agt[k] = t
                slices = [(agt[k], 0) for k in range(nko)]
                korder = [k for k in range(nko) if k % 2 == 0] + \
                         [k for k in range(nko) if k % 2 == 1]
                for it in range(jpc):
                    work_late.append(
                        lambda ci=ci, it=it, s=slices, ko=korder:
                        emit_proj(ci, it, s, ko))

            while work_early or work_late:
                pop_work(late_ok=True)

    if compile:
        nc.compile()
    return nc


def make_in_maps(x, Wq, Wk, Wv, Wo, n_cores=N_CORES):
    import ml_dtypes
    bf16 = ml_dtypes.bfloat16
    scale = np.float32(DH ** -0.5)
    # band mask for the diagonal j-tile of S^T [j,i]: keep j <= i
    mask_b = np.triu(np.ones((P, P), np.float32)).astype(bf16)
    in_maps = []
    for c in range(n_cores):
        b, r = divmod(c, 4)
        hs = r * HS
        in_maps.append({
            "xT": np.ascontiguousarray(x[b].T).astype(bf16),
            "wq": (Wq[:, hs:hs + HS] * scale).astype(bf16),
            "wk": np.ascontiguousarray(Wk[:, hs:hs + HS]).astype(bf16),
            "wv": np.ascontiguousarray(Wv[:, hs:hs + HS]).astype(bf16),
            "wo": np.ascontiguousarray(Wo[:, hs:hs + HS]).astype(bf16),
            "mask_c": mask_b,
        })
    return in_maps


def assemble_out(results, B, seq, n_cores=N_CORES):
    out = np.empty((B, seq, INNER), np.float32)
    for c in range(n_cores):
        b, r = divmod(c, 4)
        out[b][:, r * HS:(r + 1) * HS] = results[c]["out"]
    return out


_NC_CACHE = {}


def kernel(x, Wq, Wk, Wv, Wo):
    from concourse import bass_utils
    x = np.asarray(x, np.float32)
    B, seq, dim = x.shape
    key = (seq, dim)
    if key not in _NC_CACHE:
        _NC_CACHE[key] = build_nc(seq=seq, dim=dim)
    nc = _NC_CACHE[key]
    in_maps = make_in_maps(x, np.asarray(Wq, np.float32),
                           np.asarray(Wk, np.float32),
                           np.asarray(Wv, np.float32),
                           np.asarray(Wo, np.float32))
    res = bass_utils.run_bass_kernel_spmd(
        nc, in_maps, core_ids=list(range(N_CORES)))
    return assemble_out(res.results, B, seq)


# revision 15
# speedup vs baseline: 1.3743x; 1.0242x over previous
"""Distributed causal multi-head attention for one TRN2 chip (8 NeuronCores).

Sharding: batch (2) x head-groups (4 heads/core) -> 8 cores.
Core c handles batch c//4, heads [ (c%4)*4 , (c%4)*4+4 ).
Per core: QKV projections for its 4 heads, flash-style causal attention
with scores kept transposed (S^T = K @ Q^T) so the PV product needs no
transposes; V is augmented with a ones column so the softmax denominators
fall out of the same matmul (row 64 of each head's O^T psum).  Then an
AllGather of the attention output (pre-Wo, 4-core group = one batch) and
a column-sliced output projection.  Host assembles the 8 column/batch
shards.  Compute dtype bf16 (PSUM accumulation fp32), softmax in fp32.

Scheduling: the attention loop keeps the in-order PE queue saturated by
interleaving KT/V projections for later chunks and the AllGather-gated
output projections as queued work items (so the HAM clock gate never
re-throttles).  Chunk 0+1 share one AllGather (it completes under chunk
2's attention); the last chunk's AllGather is split per head-pair so the
first half overlaps pass B and the output projection can start on half
the k-tiles while the second half is still in flight.
"""

import sys
from collections import deque

import numpy as np

sys.path.insert(0, "/opt/trn_rl_repo")

import concourse.bass as bass  # noqa: E402
import concourse.bacc as bacc  # noqa: E402
import concourse.tile as tile  # noqa: E402
import concourse.mybir as mybir  # noqa: E402

F32 = mybir.dt.float32
BF16 = mybir.dt.bfloat16
ActFn = mybir.ActivationFunctionType

P = 128          # partition dim
CHUNK = 512      # i-chunk (matmul moving free dim, one psum bank of fp32)
DH = 64          # head dim
HPC = 4          # heads per core
HS = HPC * DH    # 256 per-core inner slice
DHA = DH + 1     # augmented head dim (ones column for softmax sums)
INNER = 1024     # total inner dim (16 heads x 64)
N_CORES = 8
GROUPS = [[0, 1, 2, 3], [4, 5, 6, 7]]


def build_nc(seq=2048, dim=1024, n_cores=N_CORES, groups=GROUPS, compile=True):
    """Build the SPMD Bass graph (identical on all cores)."""
    nch = seq // CHUNK          # i-chunks
    jpc = CHUNK // P            # j-tiles per chunk (4)
    njt = seq // P              # j-tiles
    nk = dim // P               # feature k-tiles
    nko = INNER // P            # inner k-tiles for the output projection
    grp = len(groups[0])        # replica group size (4)

    nc = bacc.Bacc("TRN2", target_bir_lowering=False, debug=False,
                   enable_asserts=False, num_devices=n_cores)

    xT = nc.dram_tensor("xT", [dim, seq], BF16, kind="ExternalInput").ap()
    wq = nc.dram_tensor("wq", [dim, HS], BF16, kind="ExternalInput").ap()
    wk = nc.dram_tensor("wk", [dim, HS], BF16, kind="ExternalInput").ap()
    wv = nc.dram_tensor("wv", [dim, HS], BF16, kind="ExternalInput").ap()
    wo = nc.dram_tensor("wo", [INNER, HS], BF16, kind="ExternalInput").ap()
    mask_c = nc.dram_tensor("mask_c", [P, P], BF16, kind="ExternalInput").ap()
    out = nc.dram_tensor("out", [seq, HS], F32, kind="ExternalOutput").ap()

    with tile.TileContext(nc) as tc:
        with tc.tile_pool(name="sb", bufs=1) as sb, \
             tc.tile_pool(name="ps", bufs=1, space="PSUM") as ps, \
             tc.tile_pool(name="dram", bufs=1, space="DRAM") as dram:

            # ---- load inputs ----
            xt = [sb.tile([P, seq], BF16, tag=f"xt{k}", name=f"xt{k}")
                  for k in range(nk)]
            wq_sb = [sb.tile([P, HS], BF16, tag=f"wq{k}", name=f"wq{k}")
                     for k in range(nk)]
            wk_sb = [sb.tile([P, HS], BF16, tag=f"wk{k}", name=f"wk{k}")
                     for k in range(nk)]
            wv_sb = [sb.tile([P, HS], BF16, tag=f"wv{k}", name=f"wv{k}")
                     for k in range(nk)]
            wo_sb = [sb.tile([P, HS], BF16, tag=f"wo{k}", name=f"wo{k}")
                     for k in range(nko)]
            mask_sb = sb.tile([P, P], BF16, tag="mask", name="mask")

            # inputs spread across engine DMA queues so the loads issue
            # in parallel (the sync queue alone serializes ~40 descriptors)
            for k in range(nk):
                nc.sync.dma_start(xt[k][:], xT[k * P:(k + 1) * P, :])
                nc.scalar.dma_start(wq_sb[k][:], wq[k * P:(k + 1) * P, :])
                nc.scalar.dma_start(wk_sb[k][:], wk[k * P:(k + 1) * P, :])
                nc.gpsimd.dma_start(wv_sb[k][:], wv[k * P:(k + 1) * P, :])
            for k in range(nko):
                nc.gpsimd.dma_start(wo_sb[k][:], wo[k * P:(k + 1) * P, :])
            nc.gpsimd.dma_start(mask_sb[:], mask_c[:])

            # warm up the collectives firmware while QKV runs
            warm_in = dram.tile([P, 4], BF16, tag="warm_i", name="warm_i")
            warm_out = dram.tile([grp * P, 4], BF16,
                                 tag="warm_o", name="warm_o")
            nc.sync.dma_start(warm_in[:], mask_c[0:P, 0:4])
            nc.gpsimd.collective_compute(
                "AllGather", mybir.AluOpType.bypass, replica_groups=groups,
                ins=[warm_in.opt()], outs=[warm_out.opt()])

            # persistent QKV results
            qt_sb = [sb.tile([P, seq], BF16, tag=f"qt{p}", name=f"qt{p}")
                     for p in range(2)]
            kt_sb = [sb.tile([P, seq], BF16, tag=f"kt{p}", name=f"kt{p}")
                     for p in range(2)]
            v_sb = [sb.tile([P, HPC * DHA], BF16, tag=f"v{j}", name=f"v{j}")
                    for j in range(njt)]
            ot_sb = [sb.tile([P, seq], BF16, tag=f"ot{p}", name=f"ot{p}")
                     for p in range(2)]

            # ---- interleavable work items (each emits one psum group) ----
            def emit_kt(pair, ch):
                pt = ps.tile([P, CHUNK], F32, tag="misc",
                             name=f"ktps{pair}_{ch}", bufs=2)
                for k in range(nk):
                    nc.tensor.matmul(
                        pt[:], lhsT=wk_sb[k][:, pair * P:(pair + 1) * P],
                        rhs=xt[k][:, ch * CHUNK:(ch + 1) * CHUNK],
                        start=(k == 0), stop=(k == nk - 1))
                nc.scalar.activation(
                    kt_sb[pair][:, ch * CHUNK:(ch + 1) * CHUNK], pt[:],
                    ActFn.Copy)

            def emit_v(jt):
                pt = ps.tile([P, HS], F32, tag="misc",
                             name=f"vps{jt}", bufs=2)
                for k in range(nk):
                    nc.tensor.matmul(
                        pt[:], lhsT=xt[k][:, jt * P:(jt + 1) * P],
                        rhs=wv_sb[k][:],
                        start=(k == 0), stop=(k == nk - 1))
                nc.scalar.activation(
                    v_sb[jt].rearrange("p (h d) -> p h d", h=HPC)[:, :, 0:DH],
                    pt.rearrange("p (h d) -> p h d", h=HPC), ActFn.Copy)
                nc.vector.memset(
                    v_sb[jt].rearrange("p (h d) -> p h d", h=HPC)[:, :, DH:DHA],
                    1.0)

            def emit_proj(ci, it, slices, korder):
                # one output i-tile: out[ci*CHUNK + it*128, :HS]
                # slices[k] = (ag_tile, col_offset) holding attT k-tile k
                c0 = ci * CHUNK
                op_ps = ps.tile([P, HS], F32, tag="misc",
                                name=f"op{ci}_{it}", bufs=2)
                for n, k in enumerate(korder):
                    ag_t, coff = slices[k]
                    nc.tensor.matmul(
                        op_ps[:],
                        lhsT=ag_t[:, coff + it * P:coff + (it + 1) * P],
                        rhs=wo_sb[k][:],
                        start=(n == 0), stop=(n == nko - 1))
                o_sb = sb.tile([P, HS], F32, tag="osb",
                               name=f"o{ci}_{it}", bufs=2)
                nc.vector.tensor_copy(o_sb[:], op_ps[:])
                nc.sync.dma_start(
                    out[c0 + it * P:c0 + (it + 1) * P, :], o_sb[:])

            work_early = deque()   # KT/V for future chunks (not gated)
            work_late = deque()    # output projections (gated on AllGather)

            def pop_work(late_ok):
                if work_early:
                    work_early.popleft()()
                elif late_ok and work_late:
                    work_late.popleft()()

            def emit_ag_pair(ci, pair):
                # half AllGather (one head pair) of the chunk `ci` — fired
                # right after that pair's normalize, so pair A overlaps the
                # second attention pass and both stay small (cheap on CC)
                c0 = ci * CHUNK
                bounce_in = dram.tile([P, CHUNK], BF16, tag=f"binh{pair}",
                                      name=f"binh{ci}_{pair}", bufs=2)
                bounce_out = dram.tile([grp * P, CHUNK], BF16,
                                       tag=f"bouth{pair}",
                                       name=f"bouth{ci}_{pair}", bufs=2)
                nc.sync.dma_start(bounce_in[:], ot_sb[pair][:, c0:c0 + CHUNK])
                nc.gpsimd.collective_compute(
                    "AllGather", mybir.AluOpType.bypass,
                    replica_groups=groups,
                    ins=[bounce_in.opt()], outs=[bounce_out.opt()])
                return bounce_out

            # ---- upfront projections: all of Q, chunk-0 K, chunk-0 V ----
            # Q is k-outer / weight-stationary so it pipelines with the xT
            # DMAs; two chunks share one 2-bank psum tile.
            for pair in range(2):
                for chh in range(0, nch, 2):
                    wch = min(2, nch - chh)
                    pt = ps.tile([P, wch * CHUNK], F32, tag="s2",
                                 name=f"qps{pair}_{chh}", bufs=2)
                    for k in range(nk):
                        for c in range(wch):
                            nc.tensor.matmul(
                                pt[:, c * CHUNK:(c + 1) * CHUNK],
                                lhsT=wq_sb[k][:, pair * P:(pair + 1) * P],
                                rhs=xt[k][:, (chh + c) * CHUNK:
                                           (chh + c + 1) * CHUNK],
                                start=(k == 0), stop=(k == nk - 1))
                    nc.scalar.activation(
                        qt_sb[pair][:, chh * CHUNK:(chh + wch) * CHUNK],
                        pt[:], ActFn.Copy)
            for pair in range(2):
                emit_kt(pair, 0)
            for jt in range(jpc):
                emit_v(jt)

            # ---- attention chunks ----
            last_parts = {}
            for ci in range(nch):
                jt_end = jpc * (ci + 1)
                c0 = ci * CHUNK
                last = ci == nch - 1

                if ci + 1 < nch:
                    for pair in range(2):
                        work_early.append(
                            lambda pair=pair, ch=ci + 1: emit_kt(pair, ch))
                    for jt in range(jpc * (ci + 1), jpc * (ci + 2)):
                        work_early.append(lambda jt=jt: emit_v(jt))

                for hpass in range(2):
                    # heads 2*hpass, 2*hpass+1  (== head pair `hpass`)
                    ot_ps = [ps.tile([DHA, CHUNK], F32, tag=f"ot{h2}",
                                     name=f"ot{ci}_{hpass}_{h2}", bufs=1)
                             for h2 in range(2)]
                    for jt in range(jt_end):
                        rel = max(0, (jt - jpc * ci)) * P
                        diag = jt >= jpc * ci

                        s2 = ps.tile([P, 2 * CHUNK], F32, tag="s2",
                                     name=f"s{ci}_{hpass}_{jt}", bufs=2)
                        es = sb.tile([P, 2 * CHUNK], BF16, tag="es",
                                     name=f"es{ci}_{hpass}_{jt}", bufs=3)

                        for h2 in range(2):
                            # S^T tile = K_h @ Q_h^T (row-tiled, K=64)
                            nc.tensor.matmul(
                                s2[:, h2 * CHUNK + rel:(h2 + 1) * CHUNK],
                                lhsT=kt_sb[hpass][h2 * DH:(h2 + 1) * DH,
                                                  jt * P:(jt + 1) * P],
                                rhs=qt_sb[hpass][h2 * DH:(h2 + 1) * DH,
                                                 c0 + rel:c0 + CHUNK],
                                start=True, stop=True,
                                tile_position=(h2 * DH, 0))
                        # one exp for both heads (both psum banks)
                        nc.scalar.activation(
                            es.rearrange("p (t c) -> p t c", t=2)[:, :, rel:],
                            s2.rearrange("p (t c) -> p t c", t=2)[:, :, rel:],
                            ActFn.Exp)
                        if diag:
                            for h2 in range(2):
                                nc.vector.tensor_mul(
                                    es[:, h2 * CHUNK + rel:
                                       h2 * CHUNK + rel + P],
                                    es[:, h2 * CHUNK + rel:
                                       h2 * CHUNK + rel + P],
                                    mask_sb[:])
                        for h2 in range(2):
                            h = 2 * hpass + h2
                            # O^T(+sums) accumulation: V_aug^T @ expS^T
                            nc.tensor.matmul(
                                ot_ps[h2][:, rel:CHUNK],
                                lhsT=v_sb[jt][:, h * DHA:(h + 1) * DHA],
                                rhs=es[:, h2 * CHUNK + rel:(h2 + 1) * CHUNK],
                                start=(jt == 0), stop=(jt == jt_end - 1))
                        pop_work(late_ok=(jt >= jt_end - 2))

                    # evacuate psum fast, normalize off the PE critical path
                    for h2 in range(2):
                        h = 2 * hpass + h2
                        otr = sb.tile([DH, CHUNK], F32, tag=f"otr{h2}",
                                      name=f"otr{ci}_{h}", bufs=2)
                        srow = sb.tile([1, CHUNK], F32, tag=f"sr{h2}",
                                       name=f"sr{ci}_{h}", bufs=2)
                        nc.vector.tensor_copy(otr[:], ot_ps[h2][0:DH, :])
                        nc.vector.tensor_copy(srow[:], ot_ps[h2][DH:DHA, :])
                        rcp = sb.tile([1, CHUNK], F32, tag=f"rcp{h2}",
                                      name=f"rcp{ci}_{h}", bufs=2)
                        nc.vector.reciprocal_approx_fast(rcp[:], srow[:])
                        bc_sb = sb.tile([DH, CHUNK], F32, tag=f"bc{h2}",
                                        name=f"bc{ci}_{h}", bufs=2)
                        nc.gpsimd.partition_broadcast(bc_sb[:], rcp[:],
                                                      channels=DH)
                        nc.vector.tensor_mul(
                            ot_sb[hpass][h2 * DH:(h2 + 1) * DH,
                                         c0:c0 + CHUNK],
                            otr[:], bc_sb[:])

                    # this pair's half of the chunk goes out now
                    last_parts[hpass] = emit_ag_pair(ci, hpass)

                # gathered attT loads (after both bounce writes, so the sync
                # queue is past every producer), evens then odds; then the
                # gated output projections
                agt = {}
                for pair in range(2):
                    bo = last_parts[pair]
                    for r in range(grp):
                        k = 2 * r + pair
                        t = sb.tile([P, CHUNK], BF16, tag=f"ag{k}",
                                    name=f"ag{ci}_{k}", bufs=2)
                        nc.sync.dma_start(t[:], bo[r * P:(r + 1) * P, :])
                        agt[k] = t
                slices = [(agt[k], 0) for k in range(nko)]
                korder = [k for k in range(nko) if k % 2 == 0] + \
                         [k for k in range(nko) if k % 2 == 1]
                for it in range(jpc):
                    work_late.append(
                        lambda ci=ci, it=it, s=slices, ko=korder:
                        emit_proj(ci, it, s, ko))

            while work_early or work_late:
                pop_work(late_ok=True)

    if compile:
        nc.compile()
    return nc


def make_in_maps(x, Wq, Wk, Wv, Wo, n_cores=N_CORES):
    import ml_dtypes
    bf16 = ml_dtypes.bfloat16
    scale = np.float32(DH ** -0.5)
    # band mask for the diagonal j-tile of S^T [j,i]: keep j <= i
    mask_b = np.triu(np.ones((P, P), np.float32)).astype(bf16)
    in_maps = []
    for c in range(n_cores):
        b, r = divmod(c, 4)
        hs = r * HS
        in_maps.append({
            "xT": np.ascontiguousarray(x[b].T).astype(bf16),
            "wq": (Wq[:, hs:hs + HS] * scale).astype(bf16),
            "wk": np.ascontiguousarray(Wk[:, hs:hs + HS]).astype(bf16),
            "wv": np.ascontiguousarray(Wv[:, hs:hs + HS]).astype(bf16),
            "wo": np.ascontiguousarray(Wo[:, hs:hs + HS]).astype(bf16),
            "mask_c": mask_b,
        })
    return in_maps


def assemble_out(results, B, seq, n_cores=N_CORES):
    out = np.empty((B, seq, INNER), np.float32)
    for c in range(n_cores):
        b, r = divmod(c, 4)
        out[b][:, r * HS:(r + 1) * HS] = results[c]["out"]
    return out


_NC_CACHE = {}


def kernel(x, Wq, Wk, Wv, Wo):
    from concourse import bass_utils
    x = np.asarray(x, np.float32)
    B, seq, dim = x.shape
    key = (seq, dim)
    if key not in _NC_CACHE:
        _NC_CACHE[key] = build_nc(seq=seq, dim=dim)
    nc = _NC_CACHE[key]
    in_maps = make_in_maps(x, np.asarray(Wq, np.float32),
                           np.asarray(Wk, np.float32),
                           np.asarray(Wv, np.float32),
                           np.asarray(Wo, np.float32))
    res = bass_utils.run_bass_kernel_spmd(
        nc, in_maps, core_ids=list(range(N_CORES)))
    return assemble_out(res.results, B, seq)
